# revision 1
# baseline (speedup 1.0000x reference)
"""Trainium2 Bass kernel for nn_GCLSTM (B=512, T=32, H=300, 10 neighbors).

Sharding: T is split across 8 cores (4 timesteps each). The reference's
scan carry (`con`) is a pure function of the per-step input, so every
timestep's cells() output can be computed independently; each core only
additionally computes cells() for its t0-1 block to get `lastcon`.
The flat-reshape softmax scramble mixes the whole batch at fixed t, so
keeping B intact per core makes it core-local.

The t=0 step (core 0 only) uses con0 directly instead of
relu(wp@[con;lastcon]+bp); this is folded into the uniform SPMD program
by giving core 0 identity/zero weights for its tau=0 con1 matmul and a
+C/-C relu trick (C=2 on core 0 -> passthrough).

PE columns are the bottleneck at this box's fixed 1.2 GHz PE clock, so
the three big GEMMs (con1: 600-dim contraction, fc1: 300/301-dim)
run in fp8e4 with DoubleRow perf mode: two 128-row K-tiles per pass,
halving column count. con/con1/htar intermediates are stored fp8 in
DR-plane-friendly layouts. Simulated end-to-end fp8 error: 6e-4
(budget 2e-2).
"""
import os
import sys

for _p in ("/opt/trn_rl_repo", "/root/.axon_site", "/root/.axon_site/_ro/trn_rl_repo",
           "/root/.axon_site/_ro/pypackages"):
    if os.path.isdir(_p) and _p not in sys.path:
        sys.path.append(_p)

import numpy as np
import ml_dtypes
from contextlib import ExitStack

import concourse.bass as bass
import concourse.tile as tile
from concourse.tile_rust import add_dep_helper
from concourse import bacc, mybir
from concourse import bass_utils
from concourse.bass_interp import get_hw_module

BF16 = mybir.dt.bfloat16
F32 = mybir.dt.float32
FP8 = mybir.dt.float8e4
NPBF = ml_dtypes.bfloat16
NPF8 = ml_dtypes.float8_e4m3
AF = mybir.ActivationFunctionType
ALU = mybir.AluOpType
DR = mybir.MatmulPerfMode.DoubleRow

B, T, H = 512, 32, 300
NCORES = 8
TLOC = T // NCORES            # 4 timesteps per core
R = TLOC * B                  # 2048 rows per core (tau*512 + b)
NK = 10                       # neighbors
HC = [(0, 128), (128, 128), (256, 44)]   # H=300 chunking (offset, size)
OFF2 = [0, 256, 512]          # per-oc col offset into DR-packed wp tiles
MSZ = [128, 128, 64]          # DR matmul M per oc (oc2 zero-padded 44->64)
NRT = R // 128                # 16 row-tiles of 128
# tau' cell blocks: 5 per core (t0-1 .. t0+3), processed in pairs
TPAIRS = [(0, 1), (2, 3), (4,)]


def _scalars_key(sc):
    return tuple(float(v) for v in sc)


_BUILD_CACHE = {}


def _build(f2_8, f2a, b2s, c0, hw=True):
    nc = bacc.Bacc("TRN2", target_bir_lowering=False, debug=False,
                   enable_asserts=False, num_devices=NCORES)

    def din(name, shape, dt):
        return nc.dram_tensor(name, shape, dt, kind="ExternalInput").ap()

    # --- per-core data inputs ---
    xT = din("xT", [NK, 12, 5 * B], BF16)        # [k, i(11)+ones, tau'*512+b]
    xrow = din("xrow", [R, 190], F32)            # fl8(10) fl10(10) xfus(170)
    exT = din("exT", [5, R], BF16)               # extras features + ones row
    ones8 = din("ones8", [1, R], FP8)            # fp8 ones row (htar c2 b1 trick)
    # --- weights (replicated; wp*0/bp0/subC differ on core 0) ---
    wihT = din("wihT", [NK, 12, 900], BF16)      # per-k lstm weights + bias row
    wtT = din("wtT", [5, 900], BF16)             # target-cell weights + bias row
    # DR-packed wp (fp8): per-oc [2, osz] blocks at OFF2[oc]
    wpAn = din("wpAn", [128, 640], FP8)          # con (c0,c1) K-tile planes
    wpBn = din("wpBn", [128, 640], FP8)          # lastcon (c0,c1) planes
    wpCn = din("wpCn", [108, 640], FP8)          # (lc c2, con c2) planes @p0 and @p64
    wpA0 = din("wpA0", [128, 640], FP8)
    wpB0 = din("wpB0", [128, 640], FP8)
    wpC0 = din("wpC0", [108, 640], FP8)
    bpn = din("bpn", [128, 3], F32)              # bp per oc chunk (partition p)
    bp0C = din("bp0C", [128, 3], F32)            # core0: +C, else bp
    subC = din("subC", [128, 1], F32)            # core0: C, else 0
    f1aDR = din("f1aDR", [128, 404], FP8)        # [v1|F1top|0pad] rows (0:128, 128:256)
    f1a2 = din("f1a2", [44, 202], FP8)           # rows 256:300 (+0pad col)
    f1bDR = din("f1bDR", [128, 404], FP8)        # [0|F1bot|0pad] rows (0:128, 128:256)
    f1b2 = din("f1b2", [45, 202], FP8)           # rows 256:300 + b1 ones-row (+0pad)
    f2full = din("f2full", [128, 200], F32)
    w3full = din("w3full", [128, 170], F32)
    afull = din("afull", [128, 10], F32)
    # --- outputs / scratch ---
    preds = nc.dram_tensor("preds", [NRT, 128], F32, kind="ExternalOutput").ap()
    smflat = [nc.dram_tensor(f"smflat{t}", [B * NK], F32, kind="Internal").ap()
              for t in range(TLOC)]
    # write view: [p, i, k] -> flat k*512 + i*128 + p  (one DMA per tau)
    smW = [s.rearrange("(k i p) -> p i k", k=NK, i=4, p=128) for s in smflat]
    # read view: [p, i, j] -> flat (i*128+p)*10 + j  (one DMA per tau)
    smR = [s.rearrange("(i p j) -> p i j", i=4, p=128, j=NK) for s in smflat]
    predsW = preds.rearrange("(t i) p -> t p i", t=TLOC, i=4)

    with tile.TileContext(nc) as tc:
        with ExitStack() as ctx:
            wpool = ctx.enter_context(tc.tile_pool(name="wpool", bufs=1))
            conp = ctx.enter_context(tc.tile_pool(name="conp", bufs=1))
            con1p = ctx.enter_context(tc.tile_pool(name="con1p", bufs=2))
            htarp = ctx.enter_context(tc.tile_pool(name="htarp", bufs=1))
            scrp = ctx.enter_context(tc.tile_pool(name="scrp", bufs=3))
            rowp = ctx.enter_context(tc.tile_pool(name="rowp", bufs=1))
            smallp = ctx.enter_context(tc.tile_pool(name="smallp", bufs=2))
            xkp = ctx.enter_context(tc.tile_pool(name="xkp", bufs=1))
            xrp = ctx.enter_context(tc.tile_pool(name="xrp", bufs=2))
            t0p = ctx.enter_context(tc.tile_pool(name="t0p", bufs=1))
            psA = ctx.enter_context(tc.tile_pool(name="psA", bufs=1, space="PSUM"))
            psB = ctx.enter_context(tc.tile_pool(name="psB", bufs=1, space="PSUM"))
            psC = ctx.enter_context(tc.tile_pool(name="psC", bufs=1, space="PSUM"))

            # ---------------- phase W: load weights/constants ----------------
            def wtile(name, shape, dt, src):
                t = wpool.tile(shape, dt, tag=name)
                nc.sync.dma_start(t[:], src)
                return t

            wt_sb = wtile("wtT", [5, 900], BF16, wtT[:])
            ex_sb = wtile("exT", [5, R], BF16, exT[:])
            wih_sb = [wtile(f"wih{k}", [12, 900], BF16, xTsrc)
                      for k, xTsrc in ((k, wihT[k]) for k in range(NK))]
            wpA_sb = wtile("wpAn", [128, 640], FP8, wpAn[:])
            wpB_sb = wtile("wpBn", [128, 640], FP8, wpBn[:])
            wpC_sb = wtile("wpCn", [108, 640], FP8, wpCn[:])
            wpA0_sb = wtile("wpA0", [128, 640], FP8, wpA0[:])
            wpB0_sb = wtile("wpB0", [128, 640], FP8, wpB0[:])
            wpC0_sb = wtile("wpC0", [108, 640], FP8, wpC0[:])
            f1a_sb = wtile("f1aDR", [128, 404], FP8, f1aDR[:])
            f1a2_sb = wtile("f1a2", [44, 202], FP8, f1a2[:])
            f1b_sb = wtile("f1bDR", [128, 404], FP8, f1bDR[:])
            f1b2_sb = wtile("f1b2", [45, 202], FP8, f1b2[:])
            f2_sb = wtile("f2full", [128, 200], F32, f2full[:])
            w3_sb = wtile("w3full", [128, 170], F32, w3full[:])
            a_sb = wtile("afull", [128, 10], F32, afull[:])
            bpn_sb = wtile("bpn", [128, 3], F32, bpn[:])
            bp0_sb = wtile("bp0C", [128, 3], F32, bp0C[:])
            sc_sb = wtile("subC", [128, 1], F32, subC[:])

            GOFF = {"i": 0, "g": 300, "o": 600}
            # static PSUM tiles: pool.tile() per group costs ~1us of
            # TileRelease semaphore latency on the PE; fixed tiles rely on
            # direct producer/consumer deps only.
            psA_t = [psA.tile([128, 1024], F32, name=f"psAs{i}", tag=f"psAs{i}")
                     for i in range(2)]
            psB_t = [psB.tile([128, 512], F32, name=f"psBs{i}", tag=f"psBs{i}")
                     for i in range(2)]
            psC_t = [psC.tile([128, 512], F32, name=f"psCs{i}", tag=f"psCs{i}")
                     for i in range(2)]
            rrA = [0]
            rrB = [0]
            rrC = [0]
            for _pt in psA_t:
                nc.vector.memset(_pt[32:64, :], 0.0)

            def nextps(tiles, rr):
                t = tiles[rr[0] % len(tiles)]
                rr[0] += 1
                return t

            def two(ap):
                return ap.rearrange("p (two n) -> p two n", two=2)

            # LSTM-cell gate pipeline: 3 matmul-evictions + product chain.
            # psum W cols wide (1 or 2 chunks of 512); rhs_fn(j) gives the
            # [K,512] moving operand for sub-chunk j; out_fn(j) the dest AP.
            def gate_chain(c, W, nsub, lhsT_fn, rhs_fn, out_fn, filler=None):
                coff, csz = HC[c]
                tiles = {}
                for g, func in (("i", AF.Sigmoid), ("g", AF.Tanh), ("o", AF.Sigmoid)):
                    if g == "o" and filler is not None:
                        filler()
                    ps = nextps(psA_t, rrA)[0:csz, 0:W]
                    for j in range(nsub):
                        nc.tensor.matmul(ps[:, j * 512:(j + 1) * 512],
                                         lhsT_fn(GOFF[g] + coff, csz),
                                         rhs_fn(j), start=True, stop=True)
                    t = scrp.tile([csz, W], BF16, name=f"sc_{g}", tag=f"sc_{g}")
                    nc.scalar.activation(t[:], ps[:], func)
                    tiles[g] = t
                m1 = scrp.tile([csz, W], BF16, name="sc_m1", tag="sc_m1")
                nc.gpsimd.tensor_mul(m1[:], tiles["i"][:], tiles["g"][:])
                m2 = scrp.tile([csz, W], BF16, name="sc_m2", tag="sc_m2")
                nc.scalar.activation(m2[:], m1[:], AF.Tanh)
                for j in range(nsub):
                    cols = slice(j * 512, (j + 1) * 512)
                    nc.gpsimd.tensor_mul(out_fn(j), tiles["o"][:, cols],
                                         m2[:, cols])

            # ---------------- phase H: target cell (htar, fp8 DR layout) ----
            # HT01: feats (c0|c1) planes at free offset 0 / 2048; HT2: c2
            # feats rows 0-43 + ones row 44 (b1 bias trick).
            HT01 = htarp.tile([128, 4096], FP8, tag="HT01")
            HT2 = htarp.tile([45, 2048], FP8, tag="HT2")
            nc.sync.dma_start(HT2[44:45, :], ones8[:])
            for ccp in ((0, 1), (2, 3)):
                for c in range(3):
                    def ht_out(j, c=c, ccp=ccp):
                        colb = (ccp[0] + j) * 512
                        if c < 2:
                            return HT01[0:128, c * 2048 + colb:c * 2048 + colb + 512]
                        return HT2[0:44, colb:colb + 512]
                    gate_chain(
                        c, 512 * len(ccp), len(ccp),
                        lambda off, sz: wt_sb[:, off:off + sz],
                        lambda j: ex_sb[:, (ccp[0] + j) * 512:(ccp[0] + j + 1) * 512],
                        ht_out)

            # con storage (fp8): CA[k] [128, 5*1024] = per-taup (c0|c1)
            # 512-blocks; C2[k] [44, 5*512] taup-major. Adjacent free blocks
            # give the DoubleRow K-tile plane pairs directly.
            CA = [conp.tile([128, 5 * 1024], FP8, name=f"CA{k}", tag=f"CA{k}")
                  for k in range(NK)]
            # c2 feats for k-pairs: even k at rows 0-43, odd k at rows 64-107
            # (rows 44-63 are dead; zeroed in psum once below)
            C2P = [conp.tile([108, 5 * 512], FP8, name=f"C2P{kp}", tag=f"C2P{kp}")
                   for kp in range(NK // 2)]

            def emit_cells_chain01(pi, k, xk1, fillers=None):
                tp = TPAIRS[pi]
                W = 512 * len(tp)

                def cells_out(j, c):
                    taup = tp[j]
                    if c == 0:
                        return CA[k][0:128, taup * 1024:taup * 1024 + 512]
                    return CA[k][0:128, taup * 1024 + 512:(taup + 1) * 1024]

                for c in range(2):
                    gate_chain(
                        c, W, len(tp),
                        lambda off, sz, k=k: wih_sb[k][:, off:off + sz],
                        lambda j, xk1=xk1: xk1[:, j * 512:(j + 1) * 512],
                        lambda j, c=c: cells_out(j, c),
                        filler=(fillers[c] if fillers else None))

            def emit_c2pair_chain(pi, k0, k1, xk0, xk1, filler=None):
                # c2 gate chains of neighbors k0(rows 0-43) and k1(rows 64-107)
                # share psum + eviction instrs: full-lane ACT/GP instead of 44.
                tp = TPAIRS[pi]
                W = 512 * len(tp)
                coff = 256
                tiles = {}
                for g, func in (("i", AF.Sigmoid), ("g", AF.Tanh), ("o", AF.Sigmoid)):
                    if g == "o" and filler is not None:
                        filler()
                    ps = nextps(psA_t, rrA)
                    for j in range(len(tp)):
                        cols = slice(j * 512, (j + 1) * 512)
                        nc.tensor.matmul(ps[0:44, cols],
                                         wih_sb[k0][:, GOFF[g] + coff:GOFF[g] + 300],
                                         xk0[:, cols], start=True, stop=True)
                        nc.tensor.matmul(ps[64:108, cols],
                                         wih_sb[k1][:, GOFF[g] + coff:GOFF[g] + 300],
                                         xk1[:, cols], start=True, stop=True)
                    t = scrp.tile([108, W], BF16, name=f"sc_{g}", tag=f"sc_{g}")
                    nc.scalar.activation(t[:], ps[0:108, 0:W], func)
                    tiles[g] = t
                m1 = scrp.tile([108, W], BF16, name="sc_m1", tag="sc_m1")
                nc.gpsimd.tensor_mul(m1[:], tiles["i"][:], tiles["g"][:])
                m2 = scrp.tile([108, W], BF16, name="sc_m2", tag="sc_m2")
                nc.scalar.activation(m2[:], m1[:], AF.Tanh)
                for j in range(len(tp)):
                    taup = tp[j]
                    cols = slice(j * 512, (j + 1) * 512)
                    nc.gpsimd.tensor_mul(
                        C2P[k0 // 2][0:108, taup * 512:(taup + 1) * 512],
                        tiles["o"][:, cols], m2[:, cols])

            srow = [rowp.tile([128, NK], F32, name=f"srow{rt}", tag=f"srow{rt}") for rt in range(NRT)]
            wdyn = [rowp.tile([128, NK], F32, name=f"wdyn{rt}", tag=f"wdyn{rt}") for rt in range(NRT)]
            wfin = [rowp.tile([128, NK], F32, name=f"wfin{rt}", tag=f"wfin{rt}") for rt in range(NRT)]
            fusc = [rowp.tile([128, 1], F32, name=f"fusc{rt}", tag=f"fusc{rt}") for rt in range(NRT)]

            con1_tiles = {}  # (tau, k) -> (ct01, ct2) fp8 tiles

            def emit_con1_group(tau, k, oc):
                # con1(tau,k,oc): relu(wp @ [con_t; con_{t-1}] + bp), fp8 DR:
                # 3 matmuls of K=256/256/88 instead of 6 of K<=128.
                wA_, wB_, wC_ = ((wpA0_sb, wpB0_sb, wpC0_sb) if tau == 0
                                 else (wpA_sb, wpB_sb, wpC_sb))
                ooff, osz = HC[oc]
                o2 = OFF2[oc]
                omm = MSZ[oc]
                ps = nextps(psB_t + psC_t, rrB)[0:omm, 0:512]
                nc.tensor.matmul(
                    ps[:], two(wA_[:, o2:o2 + 2 * omm]),
                    two(CA[k][:, (tau + 1) * 1024:(tau + 2) * 1024]),
                    start=True, stop=False, perf_mode=DR)
                nc.tensor.matmul(
                    ps[:], two(wB_[:, o2:o2 + 2 * omm]),
                    two(CA[k][:, tau * 1024:(tau + 1) * 1024]),
                    start=False, stop=False, perf_mode=DR)
                cb = 0 if k % 2 == 0 else 64
                nc.tensor.matmul(
                    ps[:], two(wC_[cb:cb + 44, o2:o2 + 2 * omm]),
                    two(C2P[k // 2][cb:cb + 44, tau * 512:(tau + 2) * 512]),
                    start=False, stop=True, perf_mode=DR)
                ps = ps[0:osz, :]
                if oc == 0:
                    ct01 = con1p.tile([128, 1024], FP8, name=f"CC01_{k}", tag=f"CC01_{k}")
                    ct2 = con1p.tile([44, 512], FP8, name=f"CC2_{k}", tag=f"CC2_{k}")
                    con1_tiles[(tau, k)] = (ct01, ct2)
                ct01, ct2 = con1_tiles[(tau, k)]
                dst = (ct01[0:128, 0:512] if oc == 0 else
                       ct01[0:128, 512:1024] if oc == 1 else
                       ct2[0:44, 0:512])
                if tau == 0:
                    # relu(psum + bp + C) - C: C=2 on core0 (identity
                    # weights feed con0 here, |con0|<1) -> exact con0
                    tmp0 = t0p.tile([osz, 512], F32, name="c1t0", tag="c1t0")
                    nc.vector.tensor_scalar(
                        tmp0[:], ps[:], bp0_sb[0:osz, oc:oc + 1], 0.0,
                        ALU.add, ALU.max)
                    nc.vector.tensor_scalar(
                        dst, tmp0[:], sc_sb[0:osz, 0:1], None,
                        ALU.subtract)
                elif tau == 2:
                    # ACT is idle in this stretch; DVE is draining fc1(1)
                    nc.scalar.activation(dst, ps[:], AF.Relu,
                                         bias=bpn_sb[0:osz, oc:oc + 1])
                else:
                    nc.vector.tensor_scalar(
                        dst, ps[:], bpn_sb[0:osz, oc:oc + 1], 0.0,
                        ALU.add, ALU.max)

            def emit_con1_k(tau, k):
                for oc in range(3):
                    emit_con1_group(tau, k, oc)

            def emit_fc1(tau):
                # fc1: Y = htar@F1bot (+b1 ones-row) once per row-tile, then
                # per-k psum += con1-part; col 0 collects s_k = v1 . con1_k.
                # Two neighbors share each psum tile. All fp8; (c0,c1) via DR.
                for rt4 in range(4):
                    rt = tau * 4 + rt4
                    colb = rt * 128
                    psY = nextps(psC_t, rrC)[:, 0:202]
                    nc.tensor.matmul(psY[:],
                                     two(HT01[:, :])[:, :, colb:colb + 128],
                                     two(f1b_sb[:]),
                                     start=True, stop=False, perf_mode=DR)
                    nc.tensor.matmul(psY[:], HT2[:, colb:colb + 128],
                                     f1b2_sb[:], start=False, stop=True)
                    ysb = smallp.tile([128, 202], F32, name="ysb", tag="ysb")
                    nc.vector.tensor_copy(ysb[:], psY[:])
                    ysb2 = ysb[:].unsqueeze(1).broadcast_to([128, 2, 202])
                    for k0 in range(0, NK, 2):
                        psF = nextps(psC_t, rrC)[:, 0:404]
                        for kk in (0, 1):
                            sl = slice(kk * 202, kk * 202 + 202)
                            ct01, ct2 = con1_tiles[(tau, k0 + kk)]
                            nc.tensor.matmul(
                                psF[:, sl],
                                two(ct01[:, :])[:, :, rt4 * 128:rt4 * 128 + 128],
                                two(f1a_sb[:]),
                                start=True, stop=False, perf_mode=DR)
                            nc.tensor.matmul(
                                psF[:, sl],
                                ct2[0:44, rt4 * 128:rt4 * 128 + 128],
                                f1a2_sb[:], start=False, stop=True)
                        f1 = smallp.tile([128, 404], F32, name="f1", tag="f1")
                        nc.vector.tensor_tensor(out=f1[:].rearrange("p (u q) -> p u q", u=2),
                                                in0=psF[:].rearrange("p (u q) -> p u q", u=2),
                                                in1=ysb2, op=ALU.add)
                        for kk in (0, 1):
                            nc.vector.tensor_copy(srow[rt][:, k0 + kk:k0 + kk + 1],
                                                  f1[:, kk * 202:kk * 202 + 1])
                            stsc = smallp.tile([128, 200], F32, name="stsc", tag="stsc")
                            nc.vector.scalar_tensor_tensor(
                                stsc[:], f1[:, kk * 202 + 1:kk * 202 + 201], 0.0,
                                f2_sb[:], ALU.max, ALU.mult,
                                accum_out=wdyn[rt][:, k0 + kk:k0 + kk + 1])

            def emit_X(tau):
                # extras: angle/fl8 wdyn terms + relu -> wfin; fusion term
                for rt4 in range(4):
                    rt = tau * 4 + rt4
                    xr = xrp.tile([128, 190], F32, name="xr", tag="xr")
                    nc.sync.dma_start(xr[:], xrow[rt * 128:rt * 128 + 128, :])
                    t1 = smallp.tile([128, 10], F32, name="t1", tag="t1")
                    nc.vector.tensor_sub(t1[:], xr[:, 10:20], a_sb[:])
                    tn = smallp.tile([128, 10], F32, name="tn", tag="tn")
                    nc.vector.tensor_scalar_mul(tn[:], t1[:], -1.0)
                    t2 = smallp.tile([128, 10], F32, name="t2", tag="t2")
                    nc.vector.tensor_max(t2[:], t1[:], tn[:])
                    o1 = smallp.tile([128, 10], F32, name="o1", tag="o1")
                    nc.vector.tensor_scalar_mul(o1[:], xr[:, 0:10], f2_8)
                    wext = smallp.tile([128, 10], F32, name="wext", tag="wext")
                    nc.vector.scalar_tensor_tensor(wext[:], t2[:], f2a, o1[:],
                                                   ALU.mult, ALU.add)
                    wsum = smallp.tile([128, 10], F32, name="wsum", tag="wsum")
                    nc.vector.tensor_add(wsum[:], wdyn[rt][:], wext[:])
                    nc.vector.tensor_scalar(wfin[rt][:], wsum[:], b2s, 0.0,
                                            ALU.add, ALU.max)
                    fsc = smallp.tile([128, 170], F32, name="fsc", tag="fsc")
                    nc.vector.scalar_tensor_tensor(
                        fsc[:], xr[:, 20:190], 0.0, w3_sb[:],
                        ALU.add, ALU.mult, accum_out=fusc[rt][:])

            # ---------------- main pipeline ----------------

            # cells chains interleaved with con1 groups at lag 2, so the PE
            # never head-of-line blocks on a chain's ACT/GP/DVE pipeline.
            def prefetch_xk(pi):
                tp = TPAIRS[pi]
                xks = []
                for k in range(NK):
                    xk1 = xkp.tile([12, 512 * len(tp)], BF16,
                                   name=f"xk{k}", tag=f"xk{k}")
                    nc.sync.dma_start(
                        xk1[:], xT[k, :, tp[0] * 512:(tp[-1] + 1) * 512])
                    xks.append(xk1)
                return xks

            def stretch(pi, tau, xks):
                for s in range(NK + 2):
                    fills = None
                    if s >= 2:
                        kf = s - 2
                        fills = [
                            (lambda kf=kf: emit_con1_group(tau, kf, 0)),
                            (lambda kf=kf: emit_con1_group(tau, kf, 1)),
                            (lambda kf=kf: emit_con1_group(tau, kf, 2)),
                        ]
                    if s < NK:
                        emit_cells_chain01(pi, s, xks[s], fillers=fills)
                        if s % 2 == 1:
                            emit_c2pair_chain(pi, s - 1, s, xks[s - 1], xks[s],
                                              filler=(fills[2] if fills else None))
                        elif fills:
                            fills[2]()
                    elif fills:
                        for f in fills:
                            f()

            def emit_S(tau):
                smtau = smallp.tile([128, 4 * NK], F32, name="smtau", tag="smtau")
                for i4 in range(4):
                    rt = tau * 4 + i4
                    e = smallp.tile([128, NK], F32, name="e", tag="e")
                    nc.scalar.activation(e[:], wfin[rt][:], AF.Exp)
                    z = smallp.tile([128, 1], F32, name="z", tag="z")
                    nc.vector.tensor_reduce(z[:], e[:], mybir.AxisListType.X, ALU.add)
                    rz = smallp.tile([128, 1], F32, name="rz", tag="rz")
                    nc.vector.reciprocal(rz[:], z[:])
                    nc.vector.tensor_scalar_mul(smtau[:, i4 * NK:(i4 + 1) * NK],
                                                e[:], rz[:, 0:1])
                for i4 in range(4):
                    nc.sync.dma_start(smW[tau][:, i4],
                                      smtau[:, i4 * NK:(i4 + 1) * NK])
                wa3t = smallp.tile([128, 4 * NK], F32, name="wa3t", tag="wa3t")
                for i4 in range(4):
                    nc.sync.dma_start(wa3t[:, i4 * NK:(i4 + 1) * NK],
                                      smR[tau][:, i4])
                prt = smallp.tile([128, 4], F32, name="prt", tag="prt")
                for i4 in range(4):
                    rt = tau * 4 + i4
                    csc = smallp.tile([128, NK], F32, name="csc", tag="csc")
                    cat = smallp.tile([128, 1], F32, name="cat", tag="cat")
                    nc.vector.scalar_tensor_tensor(
                        csc[:], srow[rt][:], 0.0,
                        wa3t[:, i4 * NK:(i4 + 1) * NK],
                        ALU.add, ALU.mult, accum_out=cat[:])
                    nc.vector.scalar_tensor_tensor(prt[:, i4:i4 + 1], cat[:], c0,
                                                   fusc[rt][:], ALU.add, ALU.add)
                nc.sync.dma_start(predsW[tau], prt[:])

            PARTS = os.environ.get("GCLSTM_PARTS", "all")

            def on(p):
                return PARTS == "all" or p in PARTS.split(",")

            xks0 = prefetch_xk(0)
            stretch(0, 0, xks0)
            emit_fc1(0)
            emit_X(0)
            xks1 = prefetch_xk(1)   # sync-queue loads ahead of S(0) DMAs
            emit_S(0)
            stretch(1, 1, xks1)
            emit_fc1(1)
            emit_X(1)
            emit_S(1)
            for k in range(NK):
                emit_con1_k(2, k)
            emit_fc1(2)
            emit_X(2)
            xks2 = prefetch_xk(2)
            emit_S(2)
            stretch(2, 3, xks2)
            emit_fc1(3)
            emit_X(3)
            emit_S(3)

    nc.compile()
    if hw:
        nc.m = get_hw_module(nc.m)
    return nc


def _get_nc(f2_8, f2a, b2s, c0):
    key = _scalars_key((f2_8, f2a, b2s, c0))
    if key not in _BUILD_CACHE:
        _BUILD_CACHE[key] = _build(f2_8, f2a, b2s, c0)
    return _BUILD_CACHE[key]


def _softmax(x):
    e = np.exp(x - x.max())
    return e / e.sum()


def _dr_pack(pl0, pl1):
    """Pack two K-tile weight planes into per-oc [2, MSZ] blocks
    (oc2 zero-padded 44->64 cols: DR ldweights rejects M=44)."""
    P = pl0.shape[0]
    out = np.zeros((P, 640), np.float32)
    for o2, omm, (ooff, osz) in zip(OFF2, MSZ, HC):
        blk = np.zeros((P, 2, omm), np.float32)
        blk[:, 0, :osz] = pl0[:, ooff:ooff + osz]
        blk[:, 1, :osz] = pl1[:, ooff:ooff + osz]
        out[:, o2:o2 + 2 * omm] = blk.reshape(P, 2 * omm)
    return out


def prepare_inputs(local_inputs, labels, extras, DisM, AngleM,
                   Wih, b_ih, b_hh, Wt, bt_ih, bt_hh,
                   wp, bp, F1, b1, F2, b2, ff, bff,
                   fuse1, biasf, Wout, biasout, a):
    """Host-side sharding + layout prep. Returns (in_maps, scalars)."""
    f = np.asarray
    local_inputs, labels, extras = f(local_inputs), f(labels), f(extras)
    aa = float(f(a)[0])
    wA = _softmax(f(DisM).astype(np.float64)).astype(np.float32)

    kept = np.r_[0:300, 600:900, 900:1200]      # gates i, g, o (f is dead)
    b_cells = (f(b_ih) + f(b_hh))[:, kept]      # [10, 900]
    bt = (f(bt_ih) + f(bt_hh))[kept]            # [900]

    # wihT[k]: [12, 900] = [Wih[k,kept,:].T ; bias row]
    wihT = np.zeros((NK, 12, 900), np.float32)
    for k in range(NK):
        wihT[k, :11] = f(Wih)[k][kept, :].T
        wihT[k, 11] = b_cells[k]
    wtT = np.zeros((5, 900), np.float32)
    wtT[:4] = f(Wt)[kept, :].T
    wtT[4] = bt

    wpT = f(wp).T.copy()                        # [600, 300]
    wpA_n = _dr_pack(wpT[0:128], wpT[128:256])
    wpB_n = _dr_pack(wpT[300:428], wpT[428:556])
    def dup64(w44):
        w = np.zeros((108, 640), np.float32)
        w[0:44] = w44
        w[64:108] = w44
        return w
    wpC_n = dup64(_dr_pack(wpT[556:600], wpT[256:300]))
    wpT0 = np.zeros((600, 300), np.float32)
    wpT0[:300] = np.eye(300, dtype=np.float32)
    wpA_0 = _dr_pack(wpT0[0:128], wpT0[128:256])
    wpB_0 = _dr_pack(wpT0[300:428], wpT0[428:556])
    wpC_0 = dup64(_dr_pack(wpT0[556:600], wpT0[256:300]))

    v1 = aa * (f(fuse1) @ f(Wout))[:, 0]        # [300]
    f1A = np.zeros((300, 202), np.float32)      # col 201 = 0 pad (even DR width)
    f1A[:, 0] = v1
    f1A[:, 1:201] = f(F1)[:300]
    f1B = np.zeros((301, 202), np.float32)
    f1B[:300, 1:201] = f(F1)[300:]
    f1B[300, 1:201] = f(b1)       # ones-row bias (htar c2 row 44)
    f1aDR = np.stack([f1A[0:128], f1A[128:256]], axis=1).reshape(128, 404)
    f1a2 = f1A[256:300]
    f1bDR = np.stack([f1B[0:128], f1B[128:256]], axis=1).reshape(128, 404)
    f1b2 = f1B[256:301]
    f2full = np.broadcast_to(f(F2)[:200, 0][None, :], (128, 200)).copy()
    afull = np.broadcast_to(f(AngleM)[None, :], (128, 10)).copy()

    ffW = (f(Wout)[:, 0] @ f(ff))               # [17]
    W3 = (1.0 - aa) * np.outer(ffW, wA)         # [17, 10]
    w3full = np.broadcast_to(W3.reshape(-1)[None, :], (128, 170)).copy()

    f2_8 = float(f(F2)[200, 0])
    f2a = float(f(F2)[201, 0]) / 360.0
    b2s = float(f(b2)[0])
    c0 = ((1.0 - aa) * float(f(Wout)[:, 0] @ f(bff)[:, 0])
          + aa * float(f(biasf) @ f(Wout)[:, 0])
          + float(f(biasout)[0]))

    bpcol = np.zeros((128, 3), np.float32)
    for oc, (ooff, osz) in enumerate(HC):
        bpcol[:osz, oc] = f(bp)[ooff:ooff + osz, 0]

    in_maps = []
    for cix in range(NCORES):
        t0 = cix * TLOC
        # x block for cells: t0-1 .. t0+3 (zeros for t=-1 on core 0)
        xblk = np.zeros((5, B, 28, NK), np.float32)
        lo = t0 - 1
        for jj in range(5):
            t = lo + jj
            if 0 <= t < T:
                xblk[jj] = local_inputs[:, t]
        xT = np.zeros((NK, 12, 5 * B), np.float32)
        # xT[k, i, taup*512+b] = xblk[taup, b, i, k]
        xT[:, :11, :] = xblk[:, :, :11, :].transpose(3, 2, 0, 1).reshape(NK, 11, 5 * B)
        xT[:, 11, :] = 1.0

        xloc = local_inputs[:, t0:t0 + TLOC]            # [B, 4, 28, 10]
        xrow = np.empty((R, 190), np.float32)
        xrow[:, 0:10] = xloc[:, :, 8, :].transpose(1, 0, 2).reshape(R, NK)
        xrow[:, 10:20] = xloc[:, :, 10, :].transpose(1, 0, 2).reshape(R, NK)
        xrow[:, 20:190] = xloc[:, :, 11:, :].transpose(1, 0, 2, 3).reshape(R, 170)

        exT = np.ones((5, R), np.float32)
        exT[:4] = f(extras)[:, t0:t0 + TLOC, :4, 0].transpose(2, 1, 0).reshape(4, R)

        core0 = cix == 0
        in_maps.append({
            "xT": xT.astype(NPBF),
            "xrow": xrow,
            "exT": exT.astype(NPBF),
            "ones8": np.ones((1, R), NPF8),
            "wihT": wihT.astype(NPBF),
            "wtT": wtT.astype(NPBF),
            "wpAn": wpA_n.astype(NPF8),
            "wpBn": wpB_n.astype(NPF8),
            "wpCn": wpC_n.astype(NPF8),
            "wpA0": (wpA_0 if core0 else wpA_n).astype(NPF8),
            "wpB0": (wpB_0 if core0 else wpB_n).astype(NPF8),
            "wpC0": (wpC_0 if core0 else wpC_n).astype(NPF8),
            "bpn": bpcol,
            "bp0C": (np.full((128, 3), 2.0, np.float32) if core0 else bpcol),
            "subC": np.full((128, 1), 2.0 if core0 else 0.0, np.float32),
            "f1aDR": f1aDR.astype(NPF8),
            "f1a2": f1a2.astype(NPF8),
            "f1bDR": f1bDR.astype(NPF8),
            "f1b2": f1b2.astype(NPF8),
            "f2full": f2full,
            "w3full": w3full,
            "afull": afull,
        })
    return in_maps, (f2_8, f2a, b2s, c0)


def kernel(local_inputs, labels, extras, DisM, AngleM,
           Wih, b_ih, b_hh, Wt, bt_ih, bt_hh,
           wp, bp, F1, b1, F2, b2, ff, bff,
           fuse1, biasf, Wout, biasout, a, _trace=False, _tmpdir=None):
    in_maps, (f2_8, f2a, b2s, c0) = prepare_inputs(
        local_inputs, labels, extras, DisM, AngleM,
        Wih, b_ih, b_hh, Wt, bt_ih, bt_hh, wp, bp, F1, b1, F2, b2,
        ff, bff, fuse1, biasf, Wout, biasout, a)
    nc = _get_nc(f2_8, f2a, b2s, c0)
    res = bass_utils.run_bass_kernel_spmd(
        nc, in_maps, core_ids=list(range(NCORES)), trace=_trace, tmpdir=_tmpdir)

    preds = np.empty((T, B, 1), np.float32)
    for cix in range(NCORES):
        out = res.results[cix]["preds"].reshape(TLOC, B)
        preds[cix * TLOC:(cix + 1) * TLOC, :, 0] = out

    labels_r = np.ascontiguousarray(
        np.transpose(np.asarray(labels), (1, 0, 2, 3)).reshape(T, B, 1))
    kernel._last_result = res
    return preds, labels_r



# revision 15
# speedup vs baseline: 1.0460x; 1.0460x over previous
"""Trainium2 Bass kernel for nn_GCLSTM (B=512, T=32, H=300, 10 neighbors).

Sharding: T is split across 8 cores (4 timesteps each). The reference's
scan carry (`con`) is a pure function of the per-step input, so every
timestep's cells() output can be computed independently; each core only
additionally computes cells() for its t0-1 block to get `lastcon`.
The flat-reshape softmax scramble mixes the whole batch at fixed t, so
keeping B intact per core makes it core-local.

The t=0 step (core 0 only) uses con0 directly instead of
relu(wp@[con;lastcon]+bp); this is folded into the uniform SPMD program
by giving core 0 identity/zero weights for its tau=0 con1 matmul and a
+C/-C relu trick (C=2 on core 0 -> passthrough).

PE columns are the bottleneck at this box's fixed 1.2 GHz PE clock, so
the three big GEMMs (con1: 600-dim contraction, fc1: 300/301-dim)
run in fp8e4 with DoubleRow perf mode: two 128-row K-tiles per pass,
halving column count. con/con1/htar intermediates are stored fp8 in
DR-plane-friendly layouts. Simulated end-to-end fp8 error: 6e-4
(budget 2e-2).
"""
import os
import sys

for _p in ("/opt/trn_rl_repo", "/root/.axon_site", "/root/.axon_site/_ro/trn_rl_repo",
           "/root/.axon_site/_ro/pypackages"):
    if os.path.isdir(_p) and _p not in sys.path:
        sys.path.append(_p)

import numpy as np
import ml_dtypes
from contextlib import ExitStack

import concourse.bass as bass
import concourse.tile as tile
from concourse.tile_rust import add_dep_helper
from concourse import bacc, mybir
from concourse import bass_utils
from concourse.bass_interp import get_hw_module

BF16 = mybir.dt.bfloat16
F32 = mybir.dt.float32
FP8 = mybir.dt.float8e4
NPBF = ml_dtypes.bfloat16
NPF8 = ml_dtypes.float8_e4m3
AF = mybir.ActivationFunctionType
ALU = mybir.AluOpType
DR = mybir.MatmulPerfMode.DoubleRow
A3 = -0.28958002765469637   # tanh(x) ~= x + A3*x^3 on |x|<=0.7 (gate range)

B, T, H = 512, 32, 300
NCORES = 8
TLOC = T // NCORES            # 4 timesteps per core
R = TLOC * B                  # 2048 rows per core (tau*512 + b)
NK = 10                       # neighbors
HC = [(0, 128), (128, 128), (256, 44)]   # H=300 chunking (offset, size)
OFF2 = [0, 256, 512]          # per-oc col offset into DR-packed wp tiles
MSZ = [128, 128, 64]          # DR matmul M per oc (oc2 zero-padded 44->64)
NRT = R // 128                # 16 row-tiles of 128
# tau' cell blocks: 5 per core (t0-1 .. t0+3), processed in pairs
TPAIRS = [(0, 1), (2, 3), (4,)]


def _scalars_key(sc):
    return tuple(float(v) for v in sc)


_BUILD_CACHE = {}


def _build(f2_8, f2a, b2s, c0, hw=True):
    nc = bacc.Bacc("TRN2", target_bir_lowering=False, debug=False,
                   enable_asserts=False, num_devices=NCORES)

    def din(name, shape, dt):
        return nc.dram_tensor(name, shape, dt, kind="ExternalInput").ap()

    # --- per-core data inputs ---
    xT = din("xT", [NK, 12, 5 * B], BF16)        # [k, i(11)+ones, tau'*512+b]
    xrow = din("xrow", [R, 190], F32)            # fl8(10) fl10(10) xfus(170)
    exT = din("exT", [5, R], BF16)               # extras features + ones row
    ones8 = din("ones8", [1, R], FP8)            # fp8 ones row (htar c2 b1 trick)
    # --- weights (replicated; wp*0/bp0/subC differ on core 0) ---
    wihT = din("wihT", [NK, 12, 900], BF16)      # per-k lstm weights + bias row
    # block-diag c2-pair weights: per pair [24, 3*108] (one matmul for both
    # neighbors' 44-dim gate chunks: rows 0:12 k0 -> out 0:44, 12:24 k1 -> 64:108)
    wihc2T = din("wihc2T", [NK // 2, 24, 324], BF16)
    wtT = din("wtT", [5, 900], BF16)             # target-cell weights + bias row
    # DR-packed wp (fp8): per-oc [2, osz] blocks at OFF2[oc]
    wpAn = din("wpAn", [128, 640], FP8)          # con (c0,c1) K-tile planes
    wpBn = din("wpBn", [128, 640], FP8)          # lastcon (c0,c1) planes
    wpCn = din("wpCn", [108, 640], FP8)          # (lc c2, con c2) planes @p0 and @p64
    wpA0 = din("wpA0", [128, 640], FP8)
    wpB0 = din("wpB0", [128, 640], FP8)
    wpC0 = din("wpC0", [108, 640], FP8)
    bpn = din("bpn", [128, 3], F32)              # bp per oc chunk (partition p)
    bp0C = din("bp0C", [128, 3], F32)            # core0: +C, else bp
    subC = din("subC", [128, 1], F32)            # core0: C, else 0
    f1aDR = din("f1aDR", [128, 404], FP8)        # [v1|F1top|0pad] rows (0:128, 128:256)
    f1a2 = din("f1a2", [44, 202], FP8)           # rows 256:300 (+0pad col)
    f1bDR = din("f1bDR", [128, 404], FP8)        # [0|F1bot|0pad] rows (0:128, 128:256)
    f1b2 = din("f1b2", [45, 202], FP8)           # rows 256:300 + b1 ones-row (+0pad)
    f2full = din("f2full", [128, 200], F32)
    w3full = din("w3full", [128, 170], F32)
    afull = din("afull", [128, 10], F32)
    # --- outputs / scratch ---
    preds = nc.dram_tensor("preds", [NRT, 128], F32, kind="ExternalOutput").ap()
    smflat = [nc.dram_tensor(f"smflat{t}", [B * NK], F32, kind="Internal").ap()
              for t in range(TLOC)]
    # write view: [p, i, k] -> flat k*512 + i*128 + p  (one DMA per tau)
    smW = [s.rearrange("(k i p) -> p i k", k=NK, i=4, p=128) for s in smflat]
    # read view: [p, i, j] -> flat (i*128+p)*10 + j  (one DMA per tau)
    smR = [s.rearrange("(i p j) -> p i j", i=4, p=128, j=NK) for s in smflat]
    predsW = preds.rearrange("(t i) p -> t p i", t=TLOC, i=4)

    with tile.TileContext(nc) as tc:
        with ExitStack() as ctx:
            wpool = ctx.enter_context(tc.tile_pool(name="wpool", bufs=1))
            conp = ctx.enter_context(tc.tile_pool(name="conp", bufs=1))
            con1p = ctx.enter_context(tc.tile_pool(name="con1p", bufs=2))
            htarp = ctx.enter_context(tc.tile_pool(name="htarp", bufs=1))
            scrp = ctx.enter_context(tc.tile_pool(name="scrp", bufs=3))
            rowp = ctx.enter_context(tc.tile_pool(name="rowp", bufs=1))
            smallp = ctx.enter_context(tc.tile_pool(name="smallp", bufs=2))
            xkp = ctx.enter_context(tc.tile_pool(name="xkp", bufs=1))
            xrp = ctx.enter_context(tc.tile_pool(name="xrp", bufs=1))
            t0p = ctx.enter_context(tc.tile_pool(name="t0p", bufs=1))
            psA = ctx.enter_context(tc.tile_pool(name="psA", bufs=1, space="PSUM"))
            psB = ctx.enter_context(tc.tile_pool(name="psB", bufs=1, space="PSUM"))
            psC = ctx.enter_context(tc.tile_pool(name="psC", bufs=1, space="PSUM"))

            # ---------------- phase W: load weights/constants ----------------
            def wtile(name, shape, dt, src):
                t = wpool.tile(shape, dt, tag=name)
                nc.sync.dma_start(t[:], src)
                return t

            wt_sb = wtile("wtT", [5, 900], BF16, wtT[:])
            ex_sb = wtile("exT", [5, R], BF16, exT[:])
            wih_sb = [wtile(f"wih{k}", [12, 900], BF16, xTsrc)
                      for k, xTsrc in ((k, wihT[k]) for k in range(NK))]
            wpA_sb = wtile("wpAn", [128, 640], FP8, wpAn[:])
            wpB_sb = wtile("wpBn", [128, 640], FP8, wpBn[:])
            wpC_sb = wtile("wpCn", [108, 640], FP8, wpCn[:])
            wpA0_sb = wtile("wpA0", [128, 640], FP8, wpA0[:])
            wpB0_sb = wtile("wpB0", [128, 640], FP8, wpB0[:])
            wpC0_sb = wtile("wpC0", [108, 640], FP8, wpC0[:])
            f1a_sb = wtile("f1aDR", [128, 404], FP8, f1aDR[:])
            f1a2_sb = wtile("f1a2", [44, 202], FP8, f1a2[:])
            f1b_sb = wtile("f1bDR", [128, 404], FP8, f1bDR[:])
            f1b2_sb = wtile("f1b2", [45, 202], FP8, f1b2[:])
            f2_sb = wtile("f2full", [128, 200], F32, f2full[:])
            w3_sb = wtile("w3full", [128, 170], F32, w3full[:])
            a_sb = wtile("afull", [128, 10], F32, afull[:])
            bpn_sb = wtile("bpn", [128, 3], F32, bpn[:])
            bp0_sb = wtile("bp0C", [128, 3], F32, bp0C[:])
            sc_sb = wtile("subC", [128, 1], F32, subC[:])
            xr_sb = []
            for rt in range(NRT):
                xrt = xrp.tile([128, 190], F32, name=f"xr{rt}", tag=f"xr{rt}")
                nc.sync.dma_start(xrt[:], xrow[rt * 128:rt * 128 + 128, :])
                xr_sb.append(xrt)

            GOFF = {"i": 0, "g": 300, "o": 600}
            # static PSUM tiles: pool.tile() per group costs ~1us of
            # TileRelease semaphore latency on the PE; fixed tiles rely on
            # direct producer/consumer deps only.
            psA_t = [psA.tile([128, 1024], F32, name=f"psAs{i}", tag=f"psAs{i}")
                     for i in range(2)]
            psB_t = [psB.tile([128, 512], F32, name=f"psBs{i}", tag=f"psBs{i}")
                     for i in range(2)]
            psC_t = [psC.tile([128, 512], F32, name=f"psCs{i}", tag=f"psCs{i}")
                     for i in range(2)]
            rrA = [0]
            rrB = [0]
            rrC = [0]
            for _pt in psA_t:
                nc.vector.memset(_pt[32:64, :], 0.0)

            def nextps(tiles, rr):
                t = tiles[rr[0] % len(tiles)]
                rr[0] += 1
                return t

            def two(ap):
                return ap.rearrange("p (two n) -> p two n", two=2)

            # LSTM-cell gate pipeline: 3 matmul-evictions + product chain.
            # psum W cols wide (1 or 2 chunks of 512); rhs_fn(j) gives the
            # [K,512] moving operand for sub-chunk j; out_fn(j) the dest AP.
            # m1 and tanh(m1) (cubic: |m1|<0.6) run on DVE to keep ACT at 3
            # table passes per chain and GPSIMD at just the fp8 out-muls.
            def dve_tanh(m1, csz, W):
                # in-place: x2 = m1^2; x2 = A3*x2+1; x2 = x2*m1  (= tanh(m1))
                x2 = scrp.tile([csz, W], BF16, name="sc_x2", tag="sc_x2")
                nc.vector.tensor_mul(x2[:], m1[:], m1[:])
                nc.vector.tensor_scalar(x2[:], x2[:], A3, 1.0, ALU.mult, ALU.add)
                nc.vector.tensor_mul(x2[:], x2[:], m1[:])
                return x2

            def gate_chain(c, W, nsub, lhsT_fn, rhs_fn, out_fn, filler=None):
                coff, csz = HC[c]
                tiles = {}
                for g, func in (("i", AF.Sigmoid), ("g", AF.Tanh), ("o", AF.Sigmoid)):
                    if g == "o" and filler is not None:
                        filler()
                    ps = nextps(psA_t, rrA)[0:csz, 0:W]
                    for j in range(nsub):
                        nc.tensor.matmul(ps[:, j * 512:(j + 1) * 512],
                                         lhsT_fn(GOFF[g] + coff, csz),
                                         rhs_fn(j), start=True, stop=True)
                    t = scrp.tile([csz, W], BF16, name=f"sc_{g}", tag=f"sc_{g}")
                    nc.scalar.activation(t[:], ps[:], func)
                    tiles[g] = t
                m1 = scrp.tile([csz, W], BF16, name="sc_m1", tag="sc_m1")
                nc.vector.tensor_mul(m1[:], tiles["i"][:], tiles["g"][:])
                m2 = dve_tanh(m1, csz, W)
                for j in range(nsub):
                    cols = slice(j * 512, (j + 1) * 512)
                    nc.gpsimd.tensor_mul(out_fn(j), tiles["o"][:, cols],
                                         m2[:, cols])

            # ---------------- phase H: target cell (htar, fp8 DR layout) ----
            # HT01: feats (c0|c1) planes at free offset 0 / 2048; HT2: c2
            # feats rows 0-43 + ones row 44 (b1 bias trick).
            HT01 = htarp.tile([128, 4096], FP8, tag="HT01")
            HT2 = htarp.tile([45, 2048], FP8, tag="HT2")
            nc.sync.dma_start(HT2[44:45, :], ones8[:])
            for ccp in ((0, 1), (2, 3)):
                for c in range(3):
                    def ht_out(j, c=c, ccp=ccp):
                        colb = (ccp[0] + j) * 512
                        if c < 2:
                            return HT01[0:128, c * 2048 + colb:c * 2048 + colb + 512]
                        return HT2[0:44, colb:colb + 512]
                    gate_chain(
                        c, 512 * len(ccp), len(ccp),
                        lambda off, sz: wt_sb[:, off:off + sz],
                        lambda j: ex_sb[:, (ccp[0] + j) * 512:(ccp[0] + j + 1) * 512],
                        ht_out)

            # con storage (fp8): CA[k] [128, 5*1024] = per-taup (c0|c1)
            # 512-blocks; C2[k] [44, 5*512] taup-major. Adjacent free blocks
            # give the DoubleRow K-tile plane pairs directly.
            CA = [conp.tile([128, 5 * 1024], FP8, name=f"CA{k}", tag=f"CA{k}")
                  for k in range(NK)]
            # c2 feats for k-pairs: even k at rows 0-43, odd k at rows 64-107
            # (rows 44-63 are dead; zeroed in psum once below)
            C2P = [conp.tile([108, 5 * 512], FP8, name=f"C2P{kp}", tag=f"C2P{kp}")
                   for kp in range(NK // 2)]

            def emit_cells_chain01(pi, k, xk1, fillers=None):
                tp = TPAIRS[pi]
                W = 512 * len(tp)

                def cells_out(j, c):
                    taup = tp[j]
                    if c == 0:
                        return CA[k][0:128, taup * 1024:taup * 1024 + 512]
                    return CA[k][0:128, taup * 1024 + 512:(taup + 1) * 1024]

                for c in range(2):
                    gate_chain(
                        c, W, len(tp),
                        lambda off, sz, k=k: wih_sb[k][:, off:off + sz],
                        lambda j, xk1=xk1: xk1[:, j * 512:(j + 1) * 512],
                        lambda j, c=c: cells_out(j, c),
                        filler=(fillers[c] if fillers else None))

            def emit_c2pair_chain(pi, k0, k1, xk0, xk1, filler=None):
                # c2 gate chains of neighbors k0(rows 0-43) and k1(rows 64-107)
                # share psum + eviction instrs: full-lane ACT/GP instead of 44.
                tp = TPAIRS[pi]
                W = 512 * len(tp)
                coff = 256
                tiles = {}
                for g, func in (("i", AF.Sigmoid), ("g", AF.Tanh), ("o", AF.Sigmoid)):
                    if g == "o" and filler is not None:
                        filler()
                    ps = nextps(psA_t, rrA)
                    for j in range(len(tp)):
                        cols = slice(j * 512, (j + 1) * 512)
                        nc.tensor.matmul(ps[0:44, cols],
                                         wih_sb[k0][:, GOFF[g] + coff:GOFF[g] + 300],
                                         xk0[:, cols], start=True, stop=True)
                        nc.tensor.matmul(ps[64:108, cols],
                                         wih_sb[k1][:, GOFF[g] + coff:GOFF[g] + 300],
                                         xk1[:, cols], start=True, stop=True)
                    t = scrp.tile([108, W], BF16, name=f"sc_{g}", tag=f"sc_{g}")
                    nc.scalar.activation(t[:], ps[0:108, 0:W], func)
                    tiles[g] = t
                m1 = scrp.tile([108, W], BF16, name="sc_m1", tag="sc_m1")
                nc.vector.tensor_mul(m1[:], tiles["i"][:], tiles["g"][:])
                m2 = dve_tanh(m1, 108, W)
                for j in range(len(tp)):
                    taup = tp[j]
                    cols = slice(j * 512, (j + 1) * 512)
                    nc.gpsimd.tensor_mul(
                        C2P[k0 // 2][0:108, taup * 512:(taup + 1) * 512],
                        tiles["o"][:, cols], m2[:, cols])

            srowT = [rowp.tile([128, 4 * NK], F32, name=f"srowT{t}", tag=f"srowT{t}")
                     for t in range(TLOC)]
            wdynT = [rowp.tile([128, 4 * NK], F32, name=f"wdynT{t}", tag=f"wdynT{t}")
                     for t in range(TLOC)]
            wfinT = [rowp.tile([128, 4 * NK], F32, name=f"wfinT{t}", tag=f"wfinT{t}")
                     for t in range(TLOC)]
            fuscT = [rowp.tile([128, 4], F32, name=f"fuscT{t}", tag=f"fuscT{t}")
                     for t in range(TLOC)]
            wa3tT = [rowp.tile([128, 4 * NK], F32, name=f"wa3tT{t}", tag=f"wa3tT{t}")
                     for t in range(TLOC)]

            con1_tiles = {}  # (tau, k) -> (ct01, ct2) fp8 tiles

            def emit_con1_group(tau, k, oc):
                # con1(tau,k,oc): relu(wp @ [con_t; con_{t-1}] + bp), fp8 DR:
                # 3 matmuls of K=256/256/88 instead of 6 of K<=128.
                wA_, wB_, wC_ = ((wpA0_sb, wpB0_sb, wpC0_sb) if tau == 0
                                 else (wpA_sb, wpB_sb, wpC_sb))
                ooff, osz = HC[oc]
                o2 = OFF2[oc]
                omm = MSZ[oc]
                ps = nextps(psB_t + psC_t, rrB)[0:omm, 0:512]
                nc.tensor.matmul(
                    ps[:], two(wA_[:, o2:o2 + 2 * omm]),
                    two(CA[k][:, (tau + 1) * 1024:(tau + 2) * 1024]),
                    start=True, stop=False, perf_mode=DR)
                nc.tensor.matmul(
                    ps[:], two(wB_[:, o2:o2 + 2 * omm]),
                    two(CA[k][:, tau * 1024:(tau + 1) * 1024]),
                    start=False, stop=False, perf_mode=DR)
                cb = 0 if k % 2 == 0 else 64
                nc.tensor.matmul(
                    ps[:], two(wC_[cb:cb + 44, o2:o2 + 2 * omm]),
                    two(C2P[k // 2][cb:cb + 44, tau * 512:(tau + 2) * 512]),
                    start=False, stop=True, perf_mode=DR)
                ps = ps[0:osz, :]
                if oc == 0:
                    ct01 = con1p.tile([128, 1024], FP8, name=f"CC01_{k}", tag=f"CC01_{k}")
                    ct2 = con1p.tile([44, 512], FP8, name=f"CC2_{k}", tag=f"CC2_{k}")
                    con1_tiles[(tau, k)] = (ct01, ct2)
                ct01, ct2 = con1_tiles[(tau, k)]
                dst = (ct01[0:128, 0:512] if oc == 0 else
                       ct01[0:128, 512:1024] if oc == 1 else
                       ct2[0:44, 0:512])
                if tau == 0:
                    # relu(psum + bp + C) - C: C=2 on core0 (identity
                    # weights feed con0 here, |con0|<1) -> exact con0
                    tmp0 = t0p.tile([osz, 512], F32, name="c1t0", tag="c1t0")
                    nc.vector.tensor_scalar(
                        tmp0[:], ps[:], bp0_sb[0:osz, oc:oc + 1], 0.0,
                        ALU.add, ALU.max)
                    nc.vector.tensor_scalar(
                        dst, tmp0[:], sc_sb[0:osz, 0:1], None,
                        ALU.subtract)
                elif tau == 2:
                    # ACT is idle in this stretch; DVE is draining fc1(1)
                    nc.scalar.activation(dst, ps[:], AF.Relu,
                                         bias=bpn_sb[0:osz, oc:oc + 1])
                else:
                    nc.vector.tensor_scalar(
                        dst, ps[:], bpn_sb[0:osz, oc:oc + 1], 0.0,
                        ALU.add, ALU.max)

            def emit_con1_k(tau, k):
                for oc in range(3):
                    emit_con1_group(tau, k, oc)

            def emit_fc1(tau):
                # fc1: Y = htar@F1bot (+b1 ones-row) once per row-tile, then
                # per-k psum += con1-part; col 0 collects s_k = v1 . con1_k.
                # Two neighbors share each psum tile. All fp8; (c0,c1) via DR.
                for rt4 in range(4):
                    rt = tau * 4 + rt4
                    colb = rt * 128
                    psY = nextps(psC_t, rrC)[:, 0:202]
                    nc.tensor.matmul(psY[:],
                                     two(HT01[:, :])[:, :, colb:colb + 128],
                                     two(f1b_sb[:]),
                                     start=True, stop=False, perf_mode=DR)
                    nc.tensor.matmul(psY[:], HT2[:, colb:colb + 128],
                                     f1b2_sb[:], start=False, stop=True)
                    ysb = smallp.tile([128, 202], F32, name="ysb", tag="ysb")
                    nc.vector.tensor_copy(ysb[:], psY[:])
                    ysb2 = ysb[:].unsqueeze(1).broadcast_to([128, 2, 202])
                    for k0 in range(0, NK, 2):
                        psF = nextps(psC_t, rrC)[:, 0:404]
                        for kk in (0, 1):
                            sl = slice(kk * 202, kk * 202 + 202)
                            ct01, ct2 = con1_tiles[(tau, k0 + kk)]
                            nc.tensor.matmul(
                                psF[:, sl],
                                two(ct01[:, :])[:, :, rt4 * 128:rt4 * 128 + 128],
                                two(f1a_sb[:]),
                                start=True, stop=False, perf_mode=DR)
                            nc.tensor.matmul(
                                psF[:, sl],
                                ct2[0:44, rt4 * 128:rt4 * 128 + 128],
                                f1a2_sb[:], start=False, stop=True)
                        f1 = smallp.tile([128, 404], F32, name="f1", tag="f1")
                        nc.vector.tensor_tensor(out=f1[:].rearrange("p (u q) -> p u q", u=2),
                                                in0=psF[:].rearrange("p (u q) -> p u q", u=2),
                                                in1=ysb2, op=ALU.add)
                        for kk in (0, 1):
                            col = rt4 * NK + k0 + kk
                            nc.vector.tensor_copy(srowT[tau][:, col:col + 1],
                                                  f1[:, kk * 202:kk * 202 + 1])
                            stsc = smallp.tile([128, 200], F32, name="stsc", tag="stsc")
                            nc.vector.scalar_tensor_tensor(
                                stsc[:], f1[:, kk * 202 + 1:kk * 202 + 201], 0.0,
                                f2_sb[:], ALU.max, ALU.mult,
                                accum_out=wdynT[tau][:, col:col + 1])

            def emit_X(tau):
                # extras: angle/fl8 wdyn terms + relu -> wfin; fusion term
                for rt4 in range(4):
                    rt = tau * 4 + rt4
                    sl = slice(rt4 * NK, (rt4 + 1) * NK)
                    xr = xr_sb[rt]
                    t1 = smallp.tile([128, 10], F32, name="t1", tag="t1")
                    nc.vector.tensor_sub(t1[:], xr[:, 10:20], a_sb[:])
                    tn = smallp.tile([128, 10], F32, name="tn", tag="tn")
                    nc.vector.tensor_scalar_mul(tn[:], t1[:], -1.0)
                    t2 = smallp.tile([128, 10], F32, name="t2", tag="t2")
                    nc.vector.tensor_max(t2[:], t1[:], tn[:])
                    o1 = smallp.tile([128, 10], F32, name="o1", tag="o1")
                    nc.vector.tensor_scalar_mul(o1[:], xr[:, 0:10], f2_8)
                    wext = smallp.tile([128, 10], F32, name="wext", tag="wext")
                    nc.vector.scalar_tensor_tensor(wext[:], t2[:], f2a, o1[:],
                                                   ALU.mult, ALU.add)
                    wsum = smallp.tile([128, 10], F32, name="wsum", tag="wsum")
                    nc.vector.tensor_add(wsum[:], wdynT[tau][:, sl], wext[:])
                    nc.vector.tensor_scalar(wfinT[tau][:, sl], wsum[:], b2s, 0.0,
                                            ALU.add, ALU.max)
                    fsc = smallp.tile([128, 170], F32, name="fsc", tag="fsc")
                    nc.vector.scalar_tensor_tensor(
                        fsc[:], xr[:, 20:190], 0.0, w3_sb[:],
                        ALU.add, ALU.mult, accum_out=fuscT[tau][:, rt4:rt4 + 1])

            # ---------------- main pipeline ----------------

            # cells chains interleaved with con1 groups at lag 2, so the PE
            # never head-of-line blocks on a chain's ACT/GP/DVE pipeline.
            def prefetch_xk(pi):
                tp = TPAIRS[pi]
                xks = []
                for k in range(NK):
                    xk1 = xkp.tile([12, 512 * len(tp)], BF16,
                                   name=f"xk{k}", tag=f"xk{k}")
                    nc.sync.dma_start(
                        xk1[:], xT[k, :, tp[0] * 512:(tp[-1] + 1) * 512])
                    xks.append(xk1)
                return xks

            def stretch(pi, tau, xks, hook=None):
                for s in range(NK + 2):
                    if hook is not None and s == hook[0]:
                        hook[1]()
                    fills = None
                    if s >= 2:
                        kf = s - 2
                        fills = [
                            (lambda kf=kf: emit_con1_group(tau, kf, 0)),
                            (lambda kf=kf: emit_con1_group(tau, kf, 1)),
                            (lambda kf=kf: emit_con1_group(tau, kf, 2)),
                        ]
                    if s < NK:
                        emit_cells_chain01(pi, s, xks[s], fillers=fills)
                        if s % 2 == 1:
                            emit_c2pair_chain(pi, s - 1, s, xks[s - 1], xks[s],
                                              filler=(fills[2] if fills else None))
                        elif fills:
                            fills[2]()
                    elif fills:
                        for f in fills:
                            f()

            def emit_S_write(tau):
                # softmax + scramble write/readback DMAs, issued early; the
                # consumer ops are queued later (emit_S_read) so the ~10us/hop
                # DMA latency never head-of-line blocks the DVE queue.
                e = smallp.tile([128, 4 * NK], F32, name="e", tag="e")
                nc.scalar.activation(e[:], wfinT[tau][:], AF.Exp)
                smtau = smallp.tile([128, 4 * NK], F32, name="smtau", tag="smtau")
                for i4 in range(4):
                    sl = slice(i4 * NK, (i4 + 1) * NK)
                    z = smallp.tile([128, 1], F32, name="z", tag="z")
                    nc.vector.tensor_reduce(z[:], e[:, sl], mybir.AxisListType.X,
                                            ALU.add)
                    rz = smallp.tile([128, 1], F32, name="rz", tag="rz")
                    nc.vector.reciprocal(rz[:], z[:])
                    nc.vector.tensor_scalar_mul(smtau[:, sl], e[:, sl], rz[:, 0:1])
                for i4 in range(4):
                    nc.sync.dma_start(smW[tau][:, i4],
                                      smtau[:, i4 * NK:(i4 + 1) * NK])
                nc.sync.dma_start(
                    wa3tT[tau][:].rearrange("p (i j) -> p i j", i=4, j=NK),
                    smR[tau][:, :, :])

            def emit_S_read(tau):
                cat4 = smallp.tile([128, 4], F32, name="cat4", tag="cat4")
                for i4 in range(4):
                    sl = slice(i4 * NK, (i4 + 1) * NK)
                    csc = smallp.tile([128, NK], F32, name="csc", tag="csc")
                    nc.vector.scalar_tensor_tensor(
                        csc[:], srowT[tau][:, sl], 0.0, wa3tT[tau][:, sl],
                        ALU.add, ALU.mult, accum_out=cat4[:, i4:i4 + 1])
                prt = smallp.tile([128, 4], F32, name="prt", tag="prt")
                nc.vector.scalar_tensor_tensor(prt[:], cat4[:], c0,
                                               fuscT[tau][:], ALU.add, ALU.add)
                nc.sync.dma_start(predsW[tau], prt[:])

            PARTS = os.environ.get("GCLSTM_PARTS", "all")

            def on(p):
                return PARTS == "all" or p in PARTS.split(",")

            xks0 = prefetch_xk(0)
            stretch(0, 0, xks0)
            emit_fc1(0)
            emit_X(0)
            xks1 = prefetch_xk(1)   # sync-queue loads ahead of S(0) DMAs
            emit_S_write(0)
            stretch(1, 1, xks1, hook=(5, lambda: emit_S_read(0)))
            emit_fc1(1)
            emit_X(1)
            emit_S_write(1)
            xks2 = prefetch_xk(2)
            for k in range(NK):
                emit_con1_k(2, k)
                if k == 4:
                    emit_S_read(1)
            emit_fc1(2)
            emit_X(2)
            emit_S_write(2)
            stretch(2, 3, xks2, hook=(5, lambda: emit_S_read(2)))
            emit_fc1(3)
            emit_X(3)
            emit_S_write(3)
            emit_S_read(3)

    nc.compile()
    if hw:
        nc.m = get_hw_module(nc.m)
    return nc


def _get_nc(f2_8, f2a, b2s, c0):
    key = _scalars_key((f2_8, f2a, b2s, c0))
    if key not in _BUILD_CACHE:
        _BUILD_CACHE[key] = _build(f2_8, f2a, b2s, c0)
    return _BUILD_CACHE[key]


def _softmax(x):
    e = np.exp(x - x.max())
    return e / e.sum()


def _dr_pack(pl0, pl1):
    """Pack two K-tile weight planes into per-oc [2, MSZ] blocks
    (oc2 zero-padded 44->64 cols: DR ldweights rejects M=44)."""
    P = pl0.shape[0]
    out = np.zeros((P, 640), np.float32)
    for o2, omm, (ooff, osz) in zip(OFF2, MSZ, HC):
        blk = np.zeros((P, 2, omm), np.float32)
        blk[:, 0, :osz] = pl0[:, ooff:ooff + osz]
        blk[:, 1, :osz] = pl1[:, ooff:ooff + osz]
        out[:, o2:o2 + 2 * omm] = blk.reshape(P, 2 * omm)
    return out


def prepare_inputs(local_inputs, labels, extras, DisM, AngleM,
                   Wih, b_ih, b_hh, Wt, bt_ih, bt_hh,
                   wp, bp, F1, b1, F2, b2, ff, bff,
                   fuse1, biasf, Wout, biasout, a):
    """Host-side sharding + layout prep. Returns (in_maps, scalars)."""
    f = np.asarray
    local_inputs, labels, extras = f(local_inputs), f(labels), f(extras)
    aa = float(f(a)[0])
    wA = _softmax(f(DisM).astype(np.float64)).astype(np.float32)

    kept = np.r_[0:300, 600:900, 900:1200]      # gates i, g, o (f is dead)
    b_cells = (f(b_ih) + f(b_hh))[:, kept]      # [10, 900]
    bt = (f(bt_ih) + f(bt_hh))[kept]            # [900]

    # wihT[k]: [12, 900] = [Wih[k,kept,:].T ; bias row]
    wihT = np.zeros((NK, 12, 900), np.float32)
    for k in range(NK):
        wihT[k, :11] = f(Wih)[k][kept, :].T
        wihT[k, 11] = b_cells[k]
    wtT = np.zeros((5, 900), np.float32)
    wtT[:4] = f(Wt)[kept, :].T
    wtT[4] = bt

    wpT = f(wp).T.copy()                        # [600, 300]
    wpA_n = _dr_pack(wpT[0:128], wpT[128:256])
    wpB_n = _dr_pack(wpT[300:428], wpT[428:556])
    def dup64(w44):
        w = np.zeros((108, 640), np.float32)
        w[0:44] = w44
        w[64:108] = w44
        return w
    wpC_n = dup64(_dr_pack(wpT[556:600], wpT[256:300]))
    wpT0 = np.zeros((600, 300), np.float32)
    wpT0[:300] = np.eye(300, dtype=np.float32)
    wpA_0 = _dr_pack(wpT0[0:128], wpT0[128:256])
    wpB_0 = _dr_pack(wpT0[300:428], wpT0[428:556])
    wpC_0 = dup64(_dr_pack(wpT0[556:600], wpT0[256:300]))

    v1 = aa * (f(fuse1) @ f(Wout))[:, 0]        # [300]
    f1A = np.zeros((300, 202), np.float32)      # col 201 = 0 pad (even DR width)
    f1A[:, 0] = v1
    f1A[:, 1:201] = f(F1)[:300]
    f1B = np.zeros((301, 202), np.float32)
    f1B[:300, 1:201] = f(F1)[300:]
    f1B[300, 1:201] = f(b1)       # ones-row bias (htar c2 row 44)
    f1aDR = np.stack([f1A[0:128], f1A[128:256]], axis=1).reshape(128, 404)
    f1a2 = f1A[256:300]
    f1bDR = np.stack([f1B[0:128], f1B[128:256]], axis=1).reshape(128, 404)
    f1b2 = f1B[256:301]
    f2full = np.broadcast_to(f(F2)[:200, 0][None, :], (128, 200)).copy()
    afull = np.broadcast_to(f(AngleM)[None, :], (128, 10)).copy()

    ffW = (f(Wout)[:, 0] @ f(ff))               # [17]
    W3 = (1.0 - aa) * np.outer(ffW, wA)         # [17, 10]
    w3full = np.broadcast_to(W3.reshape(-1)[None, :], (128, 170)).copy()

    f2_8 = float(f(F2)[200, 0])
    f2a = float(f(F2)[201, 0]) / 360.0
    b2s = float(f(b2)[0])
    c0 = ((1.0 - aa) * float(f(Wout)[:, 0] @ f(bff)[:, 0])
          + aa * float(f(biasf) @ f(Wout)[:, 0])
          + float(f(biasout)[0]))

    bpcol = np.zeros((128, 3), np.float32)
    for oc, (ooff, osz) in enumerate(HC):
        bpcol[:osz, oc] = f(bp)[ooff:ooff + osz, 0]

    in_maps = []
    for cix in range(NCORES):
        t0 = cix * TLOC
        # x block for cells: t0-1 .. t0+3 (zeros for t=-1 on core 0)
        xblk = np.zeros((5, B, 28, NK), np.float32)
        lo = t0 - 1
        for jj in range(5):
            t = lo + jj
            if 0 <= t < T:
                xblk[jj] = local_inputs[:, t]
        xT = np.zeros((NK, 12, 5 * B), np.float32)
        # xT[k, i, taup*512+b] = xblk[taup, b, i, k]
        xT[:, :11, :] = xblk[:, :, :11, :].transpose(3, 2, 0, 1).reshape(NK, 11, 5 * B)
        xT[:, 11, :] = 1.0

        xloc = local_inputs[:, t0:t0 + TLOC]            # [B, 4, 28, 10]
        xrow = np.empty((R, 190), np.float32)
        xrow[:, 0:10] = xloc[:, :, 8, :].transpose(1, 0, 2).reshape(R, NK)
        xrow[:, 10:20] = xloc[:, :, 10, :].transpose(1, 0, 2).reshape(R, NK)
        xrow[:, 20:190] = xloc[:, :, 11:, :].transpose(1, 0, 2, 3).reshape(R, 170)

        exT = np.ones((5, R), np.float32)
        exT[:4] = f(extras)[:, t0:t0 + TLOC, :4, 0].transpose(2, 1, 0).reshape(4, R)

        core0 = cix == 0
        in_maps.append({
            "xT": xT.astype(NPBF),
            "xrow": xrow,
            "exT": exT.astype(NPBF),
            "ones8": np.ones((1, R), NPF8),
            "wihT": wihT.astype(NPBF),
            "wtT": wtT.astype(NPBF),
            "wpAn": wpA_n.astype(NPF8),
            "wpBn": wpB_n.astype(NPF8),
            "wpCn": wpC_n.astype(NPF8),
            "wpA0": (wpA_0 if core0 else wpA_n).astype(NPF8),
            "wpB0": (wpB_0 if core0 else wpB_n).astype(NPF8),
            "wpC0": (wpC_0 if core0 else wpC_n).astype(NPF8),
            "bpn": bpcol,
            "bp0C": (np.full((128, 3), 2.0, np.float32) if core0 else bpcol),
            "subC": np.full((128, 1), 2.0 if core0 else 0.0, np.float32),
            "f1aDR": f1aDR.astype(NPF8),
            "f1a2": f1a2.astype(NPF8),
            "f1bDR": f1bDR.astype(NPF8),
            "f1b2": f1b2.astype(NPF8),
            "f2full": f2full,
            "w3full": w3full,
            "afull": afull,
        })
    return in_maps, (f2_8, f2a, b2s, c0)


def kernel(local_inputs, labels, extras, DisM, AngleM,
           Wih, b_ih, b_hh, Wt, bt_ih, bt_hh,
           wp, bp, F1, b1, F2, b2, ff, bff,
           fuse1, biasf, Wout, biasout, a, _trace=False, _tmpdir=None):
    in_maps, (f2_8, f2a, b2s, c0) = prepare_inputs(
        local_inputs, labels, extras, DisM, AngleM,
        Wih, b_ih, b_hh, Wt, bt_ih, bt_hh, wp, bp, F1, b1, F2, b2,
        ff, bff, fuse1, biasf, Wout, biasout, a)
    nc = _get_nc(f2_8, f2a, b2s, c0)
    res = bass_utils.run_bass_kernel_spmd(
        nc, in_maps, core_ids=list(range(NCORES)), trace=_trace, tmpdir=_tmpdir)

    preds = np.empty((T, B, 1), np.float32)
    for cix in range(NCORES):
        out = res.results[cix]["preds"].reshape(TLOC, B)
        preds[cix * TLOC:(cix + 1) * TLOC, :, 0] = out

    labels_r = np.ascontiguousarray(
        np.transpose(np.asarray(labels), (1, 0, 2, 3)).reshape(T, B, 1))
    kernel._last_result = res
    return preds, labels_r



# revision 35
# speedup vs baseline: 1.1867x; 1.1345x over previous
"""Trainium2 Bass kernel for nn_GCLSTM (B=512, T=32, H=300, 10 neighbors).

Sharding: T is split across 8 cores (4 timesteps each). The reference's
scan carry (`con`) is a pure function of the per-step input, so every
timestep's cells() output can be computed independently; each core only
additionally computes cells() for its t0-1 block to get `lastcon`.
The flat-reshape softmax scramble mixes the whole batch at fixed t, so
keeping B intact per core makes it core-local.

The t=0 step (core 0 only) uses con0 directly instead of
relu(wp@[con;lastcon]+bp); this is folded into the uniform SPMD program
by giving core 0 identity/zero weights for its tau=0 con1 matmul and a
+C/-C relu trick (C=2 on core 0 -> passthrough).

PE columns are the bottleneck at this box's fixed 1.2 GHz PE clock, so
the three big GEMMs (con1: 600-dim contraction, fc1: 300/301-dim)
run in fp8e4 with DoubleRow perf mode: two 128-row K-tiles per pass,
halving column count. con/con1/htar intermediates are stored fp8 in
DR-plane-friendly layouts. Simulated end-to-end fp8 error: 6e-4
(budget 2e-2).
"""
import os
import sys

for _p in ("/opt/trn_rl_repo", "/root/.axon_site", "/root/.axon_site/_ro/trn_rl_repo",
           "/root/.axon_site/_ro/pypackages"):
    if os.path.isdir(_p) and _p not in sys.path:
        sys.path.append(_p)

import numpy as np
import ml_dtypes
from contextlib import ExitStack

import concourse.bass as bass
import concourse.tile as tile
from concourse.tile_rust import add_dep_helper
from concourse import bacc, mybir
from concourse import bass_utils
from concourse.bass_interp import get_hw_module

BF16 = mybir.dt.bfloat16
F32 = mybir.dt.float32
FP8 = mybir.dt.float8e4
NPBF = ml_dtypes.bfloat16
NPF8 = ml_dtypes.float8_e4m3
AF = mybir.ActivationFunctionType
ALU = mybir.AluOpType
DR = mybir.MatmulPerfMode.DoubleRow
A3 = -0.28958002765469637   # tanh(x) ~= x + A3*x^3 on |x|<=0.7 (gate range)

B, T, H = 512, 32, 300
NCORES = 8
TLOC = T // NCORES            # 4 timesteps per core
R = TLOC * B                  # 2048 rows per core (tau*512 + b)
NK = 10                       # neighbors
HC = [(0, 128), (128, 128), (256, 44)]   # H=300 chunking (offset, size)
OFF2 = [0, 256, 512]          # per-oc col offset into DR-packed wp tiles
MSZ = [128, 128, 64]          # DR matmul M per oc (oc2 zero-padded 44->64)
NRT = R // 128                # 16 row-tiles of 128
# tau' cell blocks: 5 per core (t0-1 .. t0+3), processed in pairs
TPAIRS = [(0, 1), (2, 3), (4,)]


def _scalars_key(sc):
    return tuple(float(v) for v in sc)


_BUILD_CACHE = {}


def _build(f2_8, f2a, b2s, c0, hw=True):
    nc = bacc.Bacc("TRN2", target_bir_lowering=False, debug=False,
                   enable_asserts=False, num_devices=NCORES)

    def din(name, shape, dt):
        return nc.dram_tensor(name, shape, dt, kind="ExternalInput").ap()

    # --- per-core data inputs ---
    # pair-stacked features: rows 0:12 = k0 (11 feats + ones), rows 12:32
    # zero, rows 32:44 = k1. Legal matmul base partitions (0 / 32) for the
    # per-k chains and a safe K=44 block-diag c2-pair matmul.
    xT = din("xT", [NK // 2, 44, 5 * B], BF16)
    xrow = din("xrow", [R, 190], F32)            # fl8(10) fl10(10) xfus(170)
    exT = din("exT", [5, R], BF16)               # extras features + ones row
    ones8 = din("ones8", [1, R], FP8)            # fp8 ones row (htar c2 b1 trick)
    # --- weights (replicated; wp*0/bp0/subC differ on core 0) ---
    wihT = din("wihT", [NK // 2, 44, 900], BF16)  # pair-stacked lstm weights
    # block-diag c2-pair weights: per pair [44, 3*108] (one matmul for both
    # neighbors' 44-dim gate chunks: rows 0:12 k0 -> out 0:44, 32:44 k1 -> 64:108)
    wihc2T = din("wihc2T", [NK // 2, 44, 324], BF16)
    wtT = din("wtT", [5, 900], BF16)             # target-cell weights + bias row
    # DR-packed wp (fp8): per-oc [2, osz] blocks at OFF2[oc]
    wpAn = din("wpAn", [128, 640], FP8)          # con (c0,c1) K-tile planes
    wpBn = din("wpBn", [128, 640], FP8)          # lastcon (c0,c1) planes
    wpCn = din("wpCn", [108, 640], FP8)          # (lc c2, con c2) planes @p0 and @p64
    wpA0 = din("wpA0", [128, 640], FP8)
    wpB0 = din("wpB0", [128, 640], FP8)
    wpC0 = din("wpC0", [108, 640], FP8)
    bpn = din("bpn", [128, 3], F32)              # bp per oc chunk (partition p)
    bp0C = din("bp0C", [128, 3], F32)            # core0: +C, else bp
    subC = din("subC", [128, 1], F32)            # core0: C, else 0
    f1aDR = din("f1aDR", [128, 404], FP8)        # [v1|F1top|0pad] rows (0:128, 128:256)
    f1a2 = din("f1a2", [44, 202], FP8)           # rows 256:300 (+0pad col)
    f1bDR = din("f1bDR", [128, 404], FP8)        # [0|F1bot|0pad] rows (0:128, 128:256)
    f1b2 = din("f1b2", [45, 202], FP8)           # rows 256:300 + b1 ones-row (+0pad)
    f2full = din("f2full", [128, 200], F32)
    w3full = din("w3full", [128, 170], F32)
    afull = din("afull", [128, 10], F32)
    # --- outputs / scratch ---
    # preds[p, tau*4+i4] = row (i4*128+p) of timestep tau: 16B-contiguous
    # per-partition DMA packets (a [t*4+i, p] layout would scatter 4B packets)
    preds = nc.dram_tensor("preds", [128, 4 * TLOC], F32, kind="ExternalOutput").ap()
    smflat = [nc.dram_tensor(f"smflat{t}", [B * NK], F32, kind="Internal").ap()
              for t in range(TLOC)]
    # write view: [q=4k+i, p] -> flat (4k+i)*128 + p = k*512+i*128+p; written
    # from the PE-transposed [40,128] tile as 40 contiguous 512B packets
    # (direct [128,40]->flat would be 5120 scattered 4B packets, ~30us/tau)
    smWq = [s.rearrange("(q p) -> q p", q=4 * NK, p=128) for s in smflat]
    # read view: [p, i, j] -> flat (i*128+p)*10 + j  (one DMA per tau)
    smR = [s.rearrange("(i p j) -> p i j", i=4, p=128, j=NK) for s in smflat]

    with tile.TileContext(nc) as tc:
        with ExitStack() as ctx:
            wpool = ctx.enter_context(tc.tile_pool(name="wpool", bufs=1))
            conp = ctx.enter_context(tc.tile_pool(name="conp", bufs=1))
            con1p = ctx.enter_context(tc.tile_pool(name="con1p", bufs=2))
            htarp = ctx.enter_context(tc.tile_pool(name="htarp", bufs=1))
            scrp = ctx.enter_context(tc.tile_pool(name="scrp", bufs=3))
            rowp = ctx.enter_context(tc.tile_pool(name="rowp", bufs=1))
            smallp = ctx.enter_context(tc.tile_pool(name="smallp", bufs=2))
            xkp = ctx.enter_context(tc.tile_pool(name="xkp", bufs=1))
            xrp = ctx.enter_context(tc.tile_pool(name="xrp", bufs=1))
            t0p = ctx.enter_context(tc.tile_pool(name="t0p", bufs=1))
            psA = ctx.enter_context(tc.tile_pool(name="psA", bufs=1, space="PSUM"))
            psB = ctx.enter_context(tc.tile_pool(name="psB", bufs=1, space="PSUM"))
            psC = ctx.enter_context(tc.tile_pool(name="psC", bufs=1, space="PSUM"))

            # ---------------- phase W: load weights/constants ----------------
            def wtile(name, shape, dt, src):
                t = wpool.tile(shape, dt, tag=name)
                nc.sync.dma_start(t[:], src)
                return t

            wt_sb = wtile("wtT", [5, 900], BF16, wtT[:])
            ex_sb = wtile("exT", [5, R], BF16, exT[:])
            wihp_sb = [wtile(f"wihp{kp}", [44, 900], BF16, wihT[kp])
                       for kp in range(NK // 2)]
            wihc2_sb = [wtile(f"wihc2_{kp}", [44, 324], BF16, wihc2T[kp])
                        for kp in range(NK // 2)]

            def wih_k(k, off, sz):
                b = (k % 2) * 32
                return wihp_sb[k // 2][b:b + 12, off:off + sz]
            ident_sb = wpool.tile([128, 128], F32, tag="ident")
            from concourse.masks import make_identity
            make_identity(nc, ident_sb[:])
            wpA_sb = wtile("wpAn", [128, 640], FP8, wpAn[:])
            wpB_sb = wtile("wpBn", [128, 640], FP8, wpBn[:])
            wpC_sb = wtile("wpCn", [108, 640], FP8, wpCn[:])
            wpA0_sb = wtile("wpA0", [128, 640], FP8, wpA0[:])
            wpB0_sb = wtile("wpB0", [128, 640], FP8, wpB0[:])
            wpC0_sb = wtile("wpC0", [108, 640], FP8, wpC0[:])
            f1a_sb = wtile("f1aDR", [128, 404], FP8, f1aDR[:])
            f1a2_sb = wtile("f1a2", [44, 202], FP8, f1a2[:])
            f1b_sb = wtile("f1bDR", [128, 404], FP8, f1bDR[:])
            f1b2_sb = wtile("f1b2", [45, 202], FP8, f1b2[:])
            f2_sb = wtile("f2full", [128, 200], F32, f2full[:])
            w3_sb = wtile("w3full", [128, 170], F32, w3full[:])
            a_sb = wtile("afull", [128, 10], F32, afull[:])
            bpn_sb = wtile("bpn", [128, 3], F32, bpn[:])
            bp0_sb = wtile("bp0C", [128, 3], F32, bp0C[:])
            sc_sb = wtile("subC", [128, 1], F32, subC[:])
            xr_sb = []
            for rt in range(NRT):
                xrt = xrp.tile([128, 190], F32, name=f"xr{rt}", tag=f"xr{rt}")
                nc.sync.dma_start(xrt[:], xrow[rt * 128:rt * 128 + 128, :])
                xr_sb.append(xrt)

            GOFF = {"i": 0, "g": 300, "o": 600}
            # static PSUM tiles: pool.tile() per group costs ~1us of
            # TileRelease semaphore latency on the PE; fixed tiles rely on
            # direct producer/consumer deps only.
            psA_t = [psA.tile([128, 1024], F32, name=f"psAs{i}", tag=f"psAs{i}")
                     for i in range(2)]
            psB_t = [psB.tile([128, 512], F32, name=f"psBs{i}", tag=f"psBs{i}")
                     for i in range(2)]
            psC_t = [psC.tile([128, 512], F32, name=f"psCs{i}", tag=f"psCs{i}")
                     for i in range(2)]
            rrA = [0]
            rrB = [0]
            rrC = [0]
            for _pt in psA_t:
                nc.vector.memset(_pt[32:64, :], 0.0)

            def nextps(tiles, rr):
                t = tiles[rr[0] % len(tiles)]
                rr[0] += 1
                return t

            def two(ap):
                return ap.rearrange("p (two n) -> p two n", two=2)

            # LSTM-cell gate pipeline: 3 matmul-evictions + product chain.
            # psum W cols wide (1 or 2 chunks of 512); rhs_fn(j) gives the
            # [K,512] moving operand for sub-chunk j; out_fn(j) the dest AP.
            # m1 and tanh(m1) (cubic: |m1|<0.6) run on DVE to keep ACT at 3
            # table passes per chain and GPSIMD at just the fp8 out-muls.
            def dve_tanh(m1, csz, W):
                # in-place: x2 = m1^2; x2 = A3*x2+1; x2 = x2*m1  (= tanh(m1))
                x2 = scrp.tile([csz, W], BF16, name="sc_x2", tag="sc_x2")
                nc.vector.tensor_mul(x2[:], m1[:], m1[:])
                nc.vector.tensor_scalar(x2[:], x2[:], A3, 1.0, ALU.mult, ALU.add)
                nc.vector.tensor_mul(x2[:], x2[:], m1[:])
                return x2

            def gate_chain(c, W, nsub, lhsT_fn, rhs_fn, out_fn, filler=None):
                coff, csz = HC[c]
                tiles = {}
                for g, func in (("i", AF.Sigmoid), ("g", AF.Tanh), ("o", AF.Sigmoid)):
                    if g == "o" and filler is not None:
                        filler()
                    ps = nextps(psA_t, rrA)[0:csz, 0:W]
                    for j in range(nsub):
                        nc.tensor.matmul(ps[:, j * 512:(j + 1) * 512],
                                         lhsT_fn(GOFF[g] + coff, csz),
                                         rhs_fn(j), start=True, stop=True)
                    t = scrp.tile([csz, W], BF16, name=f"sc_{g}", tag=f"sc_{g}")
                    nc.scalar.activation(t[:], ps[:], func)
                    tiles[g] = t
                m1 = scrp.tile([csz, W], BF16, name="sc_m1", tag="sc_m1")
                nc.vector.tensor_mul(m1[:], tiles["i"][:], tiles["g"][:])
                m2 = dve_tanh(m1, csz, W)
                for j in range(nsub):
                    cols = slice(j * 512, (j + 1) * 512)
                    nc.gpsimd.tensor_mul(out_fn(j), tiles["o"][:, cols],
                                         m2[:, cols])

            # ---------------- phase H: target cell (htar, fp8 DR layout) ----
            # HT01: feats (c0|c1) planes at free offset 0 / 2048; HT2: c2
            # feats rows 0-43 + ones row 44 (b1 bias trick).
            HT01 = htarp.tile([128, 4096], FP8, tag="HT01")
            HT2 = htarp.tile([45, 2048], FP8, tag="HT2")
            nc.sync.dma_start(HT2[44:45, :], ones8[:])
            for ccp in ((0, 1), (2, 3)):
                for c in range(3):
                    def ht_out(j, c=c, ccp=ccp):
                        colb = (ccp[0] + j) * 512
                        if c < 2:
                            return HT01[0:128, c * 2048 + colb:c * 2048 + colb + 512]
                        return HT2[0:44, colb:colb + 512]
                    gate_chain(
                        c, 512 * len(ccp), len(ccp),
                        lambda off, sz: wt_sb[:, off:off + sz],
                        lambda j: ex_sb[:, (ccp[0] + j) * 512:(ccp[0] + j + 1) * 512],
                        ht_out)

            # con storage (fp8): CA[k] [128, 5*1024] = per-taup (c0|c1)
            # 512-blocks; C2[k] [44, 5*512] taup-major. Adjacent free blocks
            # give the DoubleRow K-tile plane pairs directly.
            CA = [conp.tile([128, 5 * 1024], FP8, name=f"CA{k}", tag=f"CA{k}")
                  for k in range(NK)]
            # c2 feats for k-pairs: even k at rows 0-43, odd k at rows 64-107
            # (rows 44-63 are dead; zeroed in psum once below)
            C2P = [conp.tile([108, 5 * 512], FP8, name=f"C2P{kp}", tag=f"C2P{kp}")
                   for kp in range(NK // 2)]

            def emit_cells_chain01(pi, k, xk1, fillers=None):
                tp = TPAIRS[pi]
                W = 512 * len(tp)

                def cells_out(j, c):
                    taup = tp[j]
                    if c == 0:
                        return CA[k][0:128, taup * 1024:taup * 1024 + 512]
                    return CA[k][0:128, taup * 1024 + 512:(taup + 1) * 1024]

                for c in range(2):
                    gate_chain(
                        c, W, len(tp),
                        lambda off, sz, k=k: wih_k(k, off, sz),
                        lambda j, xk1=xk1: xk1[:, j * 512:(j + 1) * 512],
                        lambda j, c=c: cells_out(j, c),
                        filler=(fillers[c] if fillers else None))

            def emit_c2pair_chain(pi, kp, xk2, filler=None):
                # c2 gate chains of neighbors 2kp (rows 0-43) and 2kp+1 (rows
                # 64-107): ONE block-diag matmul (K=24 stacked features) per
                # gate/j covers both, and eviction instrs run full-lane.
                tp = TPAIRS[pi]
                W = 512 * len(tp)
                tiles = {}
                for gi, (g, func) in enumerate(
                        (("i", AF.Sigmoid), ("g", AF.Tanh), ("o", AF.Sigmoid))):
                    if g == "o" and filler is not None:
                        filler()
                    ps = nextps(psA_t, rrA)
                    for j in range(len(tp)):
                        cols = slice(j * 512, (j + 1) * 512)
                        nc.tensor.matmul(ps[0:108, cols],
                                         wihc2_sb[kp][:, gi * 108:(gi + 1) * 108],
                                         xk2[:, cols], start=True, stop=True)
                    t = scrp.tile([108, W], BF16, name=f"sc_{g}", tag=f"sc_{g}")
                    nc.scalar.activation(t[:], ps[0:108, 0:W], func)
                    tiles[g] = t
                m1 = scrp.tile([108, W], BF16, name="sc_m1", tag="sc_m1")
                nc.vector.tensor_mul(m1[:], tiles["i"][:], tiles["g"][:])
                m2 = dve_tanh(m1, 108, W)
                for j in range(len(tp)):
                    taup = tp[j]
                    cols = slice(j * 512, (j + 1) * 512)
                    nc.gpsimd.tensor_mul(
                        C2P[kp][0:108, taup * 512:(taup + 1) * 512],
                        tiles["o"][:, cols], m2[:, cols])

            srowT = [rowp.tile([128, 4 * NK], F32, name=f"srowT{t}", tag=f"srowT{t}")
                     for t in range(TLOC)]
            wdynT = [rowp.tile([128, 4 * NK], F32, name=f"wdynT{t}", tag=f"wdynT{t}")
                     for t in range(TLOC)]
            wfinT = [rowp.tile([128, 4 * NK], F32, name=f"wfinT{t}", tag=f"wfinT{t}")
                     for t in range(TLOC)]
            fuscT = [rowp.tile([128, 4], F32, name=f"fuscT{t}", tag=f"fuscT{t}")
                     for t in range(TLOC)]
            wa3tT = [rowp.tile([128, 4 * NK], F32, name=f"wa3tT{t}", tag=f"wa3tT{t}")
                     for t in range(TLOC)]

            con1_tiles = {}  # (tau, k) -> (ct01, ct2) fp8 tiles

            def emit_con1_group(tau, k, oc):
                # con1(tau,k,oc): relu(wp @ [con_t; con_{t-1}] + bp), fp8 DR:
                # 3 matmuls of K=256/256/88 instead of 6 of K<=128.
                wA_, wB_, wC_ = ((wpA0_sb, wpB0_sb, wpC0_sb) if tau == 0
                                 else (wpA_sb, wpB_sb, wpC_sb))
                ooff, osz = HC[oc]
                o2 = OFF2[oc]
                omm = MSZ[oc]
                ps = nextps(psB_t + psC_t, rrB)[0:omm, 0:512]
                nc.tensor.matmul(
                    ps[:], two(wA_[:, o2:o2 + 2 * omm]),
                    two(CA[k][:, (tau + 1) * 1024:(tau + 2) * 1024]),
                    start=True, stop=False, perf_mode=DR)
                nc.tensor.matmul(
                    ps[:], two(wB_[:, o2:o2 + 2 * omm]),
                    two(CA[k][:, tau * 1024:(tau + 1) * 1024]),
                    start=False, stop=False, perf_mode=DR)
                cb = 0 if k % 2 == 0 else 64
                nc.tensor.matmul(
                    ps[:], two(wC_[cb:cb + 44, o2:o2 + 2 * omm]),
                    two(C2P[k // 2][cb:cb + 44, tau * 512:(tau + 2) * 512]),
                    start=False, stop=True, perf_mode=DR)
                ps = ps[0:osz, :]
                if oc == 0:
                    ct01 = con1p.tile([128, 1024], FP8, name=f"CC01_{k}", tag=f"CC01_{k}")
                    ct2 = con1p.tile([44, 512], FP8, name=f"CC2_{k}", tag=f"CC2_{k}")
                    con1_tiles[(tau, k)] = (ct01, ct2)
                ct01, ct2 = con1_tiles[(tau, k)]
                dst = (ct01[0:128, 0:512] if oc == 0 else
                       ct01[0:128, 512:1024] if oc == 1 else
                       ct2[0:44, 0:512])
                if tau == 0:
                    # relu(psum + bp + C) - C: C=2 on core0 (identity
                    # weights feed con0 here, |con0|<1) -> exact con0
                    tmp0 = t0p.tile([osz, 512], F32, name="c1t0", tag="c1t0")
                    nc.vector.tensor_scalar(
                        tmp0[:], ps[:], bp0_sb[0:osz, oc:oc + 1], 0.0,
                        ALU.add, ALU.max)
                    nc.vector.tensor_scalar(
                        dst, tmp0[:], sc_sb[0:osz, 0:1], None,
                        ALU.subtract)
                elif tau == 2:
                    # ACT is idle in this stretch; DVE is draining fc1(1)
                    nc.scalar.activation(dst, ps[:], AF.Relu,
                                         bias=bpn_sb[0:osz, oc:oc + 1])
                else:
                    nc.vector.tensor_scalar(
                        dst, ps[:], bpn_sb[0:osz, oc:oc + 1], 0.0,
                        ALU.add, ALU.max)

            def emit_con1_k(tau, k):
                for oc in range(3):
                    emit_con1_group(tau, k, oc)

            def emit_fc1(tau):
                # fc1: Y = htar@F1bot (+b1 ones-row) once per row-tile, then
                # per-k psum += con1-part; col 0 collects s_k = v1 . con1_k.
                # Two neighbors share each psum tile. All fp8; (c0,c1) via DR.
                for rt4 in range(4):
                    rt = tau * 4 + rt4
                    colb = rt * 128
                    psY = nextps(psC_t, rrC)[:, 0:202]
                    nc.tensor.matmul(psY[:],
                                     two(HT01[:, :])[:, :, colb:colb + 128],
                                     two(f1b_sb[:]),
                                     start=True, stop=False, perf_mode=DR)
                    nc.tensor.matmul(psY[:], HT2[:, colb:colb + 128],
                                     f1b2_sb[:], start=False, stop=True)
                    ysb = smallp.tile([128, 202], F32, name="ysb", tag="ysb")
                    nc.scalar.copy(ysb[:], psY[:])
                    ysb2 = ysb[:].unsqueeze(1).broadcast_to([128, 2, 202])
                    for k0 in range(0, NK, 2):
                        psF = nextps(psC_t, rrC)[:, 0:404]
                        for kk in (0, 1):
                            sl = slice(kk * 202, kk * 202 + 202)
                            ct01, ct2 = con1_tiles[(tau, k0 + kk)]
                            nc.tensor.matmul(
                                psF[:, sl],
                                two(ct01[:, :])[:, :, rt4 * 128:rt4 * 128 + 128],
                                two(f1a_sb[:]),
                                start=True, stop=False, perf_mode=DR)
                            nc.tensor.matmul(
                                psF[:, sl],
                                ct2[0:44, rt4 * 128:rt4 * 128 + 128],
                                f1a2_sb[:], start=False, stop=True)
                        f1 = smallp.tile([128, 404], F32, name="f1", tag="f1")
                        nc.vector.tensor_tensor(out=f1[:].rearrange("p (u q) -> p u q", u=2),
                                                in0=psF[:].rearrange("p (u q) -> p u q", u=2),
                                                in1=ysb2, op=ALU.add)
                        for kk in (0, 1):
                            col = rt4 * NK + k0 + kk
                            nc.scalar.copy(srowT[tau][:, col:col + 1],
                                           f1[:, kk * 202:kk * 202 + 1])
                            stsc = smallp.tile([128, 200], F32, name="stsc", tag="stsc")
                            nc.vector.scalar_tensor_tensor(
                                stsc[:], f1[:, kk * 202 + 1:kk * 202 + 201], 0.0,
                                f2_sb[:], ALU.max, ALU.mult,
                                accum_out=wdynT[tau][:, col:col + 1])

            def emit_X(tau):
                # extras: angle/fl8 wdyn terms + relu -> wfin; fusion term
                for rt4 in range(4):
                    rt = tau * 4 + rt4
                    sl = slice(rt4 * NK, (rt4 + 1) * NK)
                    xr = xr_sb[rt]
                    t1 = smallp.tile([128, 10], F32, name="t1", tag="t1")
                    nc.vector.tensor_sub(t1[:], xr[:, 10:20], a_sb[:])
                    tn = smallp.tile([128, 10], F32, name="tn", tag="tn")
                    nc.vector.tensor_scalar_mul(tn[:], t1[:], -1.0)
                    t2 = smallp.tile([128, 10], F32, name="t2", tag="t2")
                    nc.vector.tensor_max(t2[:], t1[:], tn[:])
                    o1 = smallp.tile([128, 10], F32, name="o1", tag="o1")
                    nc.vector.tensor_scalar_mul(o1[:], xr[:, 0:10], f2_8)
                    wext = smallp.tile([128, 10], F32, name="wext", tag="wext")
                    nc.vector.scalar_tensor_tensor(wext[:], t2[:], f2a, o1[:],
                                                   ALU.mult, ALU.add)
                    wsum = smallp.tile([128, 10], F32, name="wsum", tag="wsum")
                    nc.vector.tensor_add(wsum[:], wdynT[tau][:, sl], wext[:])
                    nc.vector.tensor_scalar(wfinT[tau][:, sl], wsum[:], b2s, 0.0,
                                            ALU.add, ALU.max)
                    fsc = smallp.tile([128, 170], F32, name="fsc", tag="fsc")
                    nc.vector.scalar_tensor_tensor(
                        fsc[:], xr[:, 20:190], 0.0, w3_sb[:],
                        ALU.add, ALU.mult, accum_out=fuscT[tau][:, rt4:rt4 + 1])

            # ---------------- main pipeline ----------------

            # cells chains interleaved with con1 groups at lag 2, so the PE
            # never head-of-line blocks on a chain's ACT/GP/DVE pipeline.
            def prefetch_xk(pi):
                # pair tiles [24, W]: rows 0:12 = k0 feats, 12:24 = k1 feats
                # (k-views feed the c0/c1 chains, the full tile the c2 pair)
                tp = TPAIRS[pi]
                cols = slice(tp[0] * 512, (tp[-1] + 1) * 512)
                xks, xkpairs = [], []
                for kp in range(NK // 2):
                    xk2 = xkp.tile([44, 512 * len(tp)], BF16,
                                   name=f"xk{kp}", tag=f"xk{kp}")
                    nc.sync.dma_start(xk2[:], xT[kp, :, cols])
                    xks.append(xk2[0:12, :])
                    xks.append(xk2[32:44, :])
                    xkpairs.append(xk2)
                return xks, xkpairs

            def stretch(pi, tau, xkboth, hook=None):
                xks, xkpairs = xkboth
                for s in range(NK + 2):
                    if hook is not None and s == hook[0]:
                        hook[1]()
                    fills = None
                    if s >= 2:
                        kf = s - 2
                        fills = [
                            (lambda kf=kf: emit_con1_group(tau, kf, 0)),
                            (lambda kf=kf: emit_con1_group(tau, kf, 1)),
                            (lambda kf=kf: emit_con1_group(tau, kf, 2)),
                        ]
                    if s < NK:
                        emit_cells_chain01(pi, s, xks[s], fillers=fills)
                        if s % 2 == 1:
                            emit_c2pair_chain(pi, s // 2, xkpairs[s // 2],
                                              filler=(fills[2] if fills else None))
                        elif fills:
                            fills[2]()
                    elif fills:
                        for f in fills:
                            f()

            def emit_S_write(tau):
                # softmax + scramble write/readback DMAs, issued early; the
                # consumer ops are queued later (emit_S_read) so the ~10us/hop
                # DMA latency never head-of-line blocks the DVE queue.
                e = smallp.tile([128, 4 * NK], F32, name="e", tag="e")
                nc.scalar.activation(e[:], wfinT[tau][:], AF.Exp)
                # smtau cols in q=4k+i order so its PE transpose [40,128] maps
                # to flat offsets q*128+p = k*512+i*128+p (contiguous write)
                smtau = smallp.tile([128, 4 * NK], F32, name="smtau", tag="smtau")
                smtV = smtau[:].rearrange("p (k i) -> p i k", k=NK, i=4)
                for i4 in range(4):
                    sl = slice(i4 * NK, (i4 + 1) * NK)
                    z = smallp.tile([128, 1], F32, name="z", tag="z")
                    nc.vector.tensor_reduce(z[:], e[:, sl], mybir.AxisListType.X,
                                            ALU.add)
                    rz = smallp.tile([128, 1], F32, name="rz", tag="rz")
                    nc.vector.reciprocal(rz[:], z[:])
                    nc.vector.tensor_scalar_mul(smtV[:, i4], e[:, sl], rz[:, 0:1])
                ps = nextps(psA_t, rrA)[0:40, 0:128]
                nc.tensor.transpose(ps, smtau[:], ident_sb[:])
                smtT = smallp.tile([40, 128], F32, name="smtT", tag="smtT")
                nc.scalar.copy(smtT[:], ps)
                nc.sync.dma_start(smWq[tau], smtT[:])
                nc.sync.dma_start(
                    wa3tT[tau][:].rearrange("p (i j) -> p i j", i=4, j=NK),
                    smR[tau][:, :, :])

            def emit_S_read(tau):
                cat4 = smallp.tile([128, 4], F32, name="cat4", tag="cat4")
                for i4 in range(4):
                    sl = slice(i4 * NK, (i4 + 1) * NK)
                    csc = smallp.tile([128, NK], F32, name="csc", tag="csc")
                    nc.vector.scalar_tensor_tensor(
                        csc[:], srowT[tau][:, sl], 0.0, wa3tT[tau][:, sl],
                        ALU.add, ALU.mult, accum_out=cat4[:, i4:i4 + 1])
                prt = smallp.tile([128, 4], F32, name="prt", tag="prt")
                nc.vector.scalar_tensor_tensor(prt[:], cat4[:], c0,
                                               fuscT[tau][:], ALU.add, ALU.add)
                nc.sync.dma_start(preds[:, tau * 4:(tau + 1) * 4], prt[:])

            PARTS = os.environ.get("GCLSTM_PARTS", "all")

            def on(p):
                return PARTS == "all" or p in PARTS.split(",")

            xks0 = prefetch_xk(0)
            stretch(0, 0, xks0)
            emit_fc1(0)
            emit_X(0)
            xks1 = prefetch_xk(1)   # sync-queue loads ahead of S(0) DMAs
            emit_S_write(0)
            stretch(1, 1, xks1, hook=(5, lambda: emit_S_read(0)))
            emit_fc1(1)
            emit_X(1)
            emit_S_write(1)
            xks2 = prefetch_xk(2)
            for k in range(NK):
                emit_con1_k(2, k)
                if k == 4:
                    emit_S_read(1)
            emit_fc1(2)
            emit_X(2)
            emit_S_write(2)
            stretch(2, 3, xks2, hook=(5, lambda: emit_S_read(2)))
            emit_fc1(3)
            emit_X(3)
            emit_S_write(3)
            emit_S_read(3)

    nc.compile()
    if hw:
        nc.m = get_hw_module(nc.m)
    return nc


def _get_nc(f2_8, f2a, b2s, c0):
    key = _scalars_key((f2_8, f2a, b2s, c0))
    if key not in _BUILD_CACHE:
        _BUILD_CACHE[key] = _build(f2_8, f2a, b2s, c0)
    return _BUILD_CACHE[key]


def _softmax(x):
    e = np.exp(x - x.max())
    return e / e.sum()


def _dr_pack(pl0, pl1):
    """Pack two K-tile weight planes into per-oc [2, MSZ] blocks
    (oc2 zero-padded 44->64 cols: DR ldweights rejects M=44)."""
    P = pl0.shape[0]
    out = np.zeros((P, 640), np.float32)
    for o2, omm, (ooff, osz) in zip(OFF2, MSZ, HC):
        blk = np.zeros((P, 2, omm), np.float32)
        blk[:, 0, :osz] = pl0[:, ooff:ooff + osz]
        blk[:, 1, :osz] = pl1[:, ooff:ooff + osz]
        out[:, o2:o2 + 2 * omm] = blk.reshape(P, 2 * omm)
    return out


def prepare_inputs(local_inputs, labels, extras, DisM, AngleM,
                   Wih, b_ih, b_hh, Wt, bt_ih, bt_hh,
                   wp, bp, F1, b1, F2, b2, ff, bff,
                   fuse1, biasf, Wout, biasout, a):
    """Host-side sharding + layout prep. Returns (in_maps, scalars)."""
    f = np.asarray
    local_inputs, labels, extras = f(local_inputs), f(labels), f(extras)
    aa = float(f(a)[0])
    wA = _softmax(f(DisM).astype(np.float64)).astype(np.float32)

    kept = np.r_[0:300, 600:900, 900:1200]      # gates i, g, o (f is dead)
    b_cells = (f(b_ih) + f(b_hh))[:, kept]      # [10, 900]
    bt = (f(bt_ih) + f(bt_hh))[kept]            # [900]

    # per-k [12, 900] = [Wih[k,kept,:].T ; bias row], pair-stacked into
    # [kp, 44, 900]: rows 0:12 = k0, rows 32:44 = k1 (12:32 zero)
    wih1 = np.zeros((NK, 12, 900), np.float32)
    for k in range(NK):
        wih1[k, :11] = f(Wih)[k][kept, :].T
        wih1[k, 11] = b_cells[k]
    wihT = np.zeros((NK // 2, 44, 900), np.float32)
    wihT[:, 0:12] = wih1[0::2]
    wihT[:, 32:44] = wih1[1::2]
    # block-diag c2-pair weights [kp, 44, 3*108]: rows 0:12 = k0 (-> out rows
    # 0:44), rows 32:44 = k1 (-> out rows 64:108); out cols 44:64 dead
    wihc2 = np.zeros((NK // 2, 44, 324), np.float32)
    for kp in range(NK // 2):
        for gi in range(3):
            wihc2[kp, 0:12, gi * 108:gi * 108 + 44] = \
                wih1[2 * kp, :, gi * 300 + 256:gi * 300 + 300]
            wihc2[kp, 32:44, gi * 108 + 64:gi * 108 + 108] = \
                wih1[2 * kp + 1, :, gi * 300 + 256:gi * 300 + 300]
    wtT = np.zeros((5, 900), np.float32)
    wtT[:4] = f(Wt)[kept, :].T
    wtT[4] = bt

    wpT = f(wp).T.copy()                        # [600, 300]
    wpA_n = _dr_pack(wpT[0:128], wpT[128:256])
    wpB_n = _dr_pack(wpT[300:428], wpT[428:556])
    def dup64(w44):
        w = np.zeros((108, 640), np.float32)
        w[0:44] = w44
        w[64:108] = w44
        return w
    wpC_n = dup64(_dr_pack(wpT[556:600], wpT[256:300]))
    wpT0 = np.zeros((600, 300), np.float32)
    wpT0[:300] = np.eye(300, dtype=np.float32)
    wpA_0 = _dr_pack(wpT0[0:128], wpT0[128:256])
    wpB_0 = _dr_pack(wpT0[300:428], wpT0[428:556])
    wpC_0 = dup64(_dr_pack(wpT0[556:600], wpT0[256:300]))

    v1 = aa * (f(fuse1) @ f(Wout))[:, 0]        # [300]
    f1A = np.zeros((300, 202), np.float32)      # col 201 = 0 pad (even DR width)
    f1A[:, 0] = v1
    f1A[:, 1:201] = f(F1)[:300]
    f1B = np.zeros((301, 202), np.float32)
    f1B[:300, 1:201] = f(F1)[300:]
    f1B[300, 1:201] = f(b1)       # ones-row bias (htar c2 row 44)
    f1aDR = np.stack([f1A[0:128], f1A[128:256]], axis=1).reshape(128, 404)
    f1a2 = f1A[256:300]
    f1bDR = np.stack([f1B[0:128], f1B[128:256]], axis=1).reshape(128, 404)
    f1b2 = f1B[256:301]
    f2full = np.broadcast_to(f(F2)[:200, 0][None, :], (128, 200)).copy()
    afull = np.broadcast_to(f(AngleM)[None, :], (128, 10)).copy()

    ffW = (f(Wout)[:, 0] @ f(ff))               # [17]
    W3 = (1.0 - aa) * np.outer(ffW, wA)         # [17, 10]
    w3full = np.broadcast_to(W3.reshape(-1)[None, :], (128, 170)).copy()

    f2_8 = float(f(F2)[200, 0])
    f2a = float(f(F2)[201, 0]) / 360.0
    b2s = float(f(b2)[0])
    c0 = ((1.0 - aa) * float(f(Wout)[:, 0] @ f(bff)[:, 0])
          + aa * float(f(biasf) @ f(Wout)[:, 0])
          + float(f(biasout)[0]))

    bpcol = np.zeros((128, 3), np.float32)
    for oc, (ooff, osz) in enumerate(HC):
        bpcol[:osz, oc] = f(bp)[ooff:ooff + osz, 0]

    in_maps = []
    for cix in range(NCORES):
        t0 = cix * TLOC
        # x block for cells: t0-1 .. t0+3 (zeros for t=-1 on core 0)
        xblk = np.zeros((5, B, 28, NK), np.float32)
        lo = t0 - 1
        for jj in range(5):
            t = lo + jj
            if 0 <= t < T:
                xblk[jj] = local_inputs[:, t]
        xk1 = np.zeros((NK, 12, 5 * B), np.float32)
        # xk1[k, i, taup*512+b] = xblk[taup, b, i, k]
        xk1[:, :11, :] = xblk[:, :, :11, :].transpose(3, 2, 0, 1).reshape(NK, 11, 5 * B)
        xk1[:, 11, :] = 1.0
        xT = np.zeros((NK // 2, 44, 5 * B), np.float32)
        xT[:, 0:12] = xk1[0::2]
        xT[:, 32:44] = xk1[1::2]

        xloc = local_inputs[:, t0:t0 + TLOC]            # [B, 4, 28, 10]
        xrow = np.empty((R, 190), np.float32)
        xrow[:, 0:10] = xloc[:, :, 8, :].transpose(1, 0, 2).reshape(R, NK)
        xrow[:, 10:20] = xloc[:, :, 10, :].transpose(1, 0, 2).reshape(R, NK)
        xrow[:, 20:190] = xloc[:, :, 11:, :].transpose(1, 0, 2, 3).reshape(R, 170)

        exT = np.ones((5, R), np.float32)
        exT[:4] = f(extras)[:, t0:t0 + TLOC, :4, 0].transpose(2, 1, 0).reshape(4, R)

        core0 = cix == 0
        in_maps.append({
            "xT": xT.astype(NPBF),
            "xrow": xrow,
            "exT": exT.astype(NPBF),
            "ones8": np.ones((1, R), NPF8),
            "wihT": wihT.astype(NPBF),
            "wihc2T": wihc2.astype(NPBF),
            "wtT": wtT.astype(NPBF),
            "wpAn": wpA_n.astype(NPF8),
            "wpBn": wpB_n.astype(NPF8),
            "wpCn": wpC_n.astype(NPF8),
            "wpA0": (wpA_0 if core0 else wpA_n).astype(NPF8),
            "wpB0": (wpB_0 if core0 else wpB_n).astype(NPF8),
            "wpC0": (wpC_0 if core0 else wpC_n).astype(NPF8),
            "bpn": bpcol,
            "bp0C": (np.full((128, 3), 2.0, np.float32) if core0 else bpcol),
            "subC": np.full((128, 1), 2.0 if core0 else 0.0, np.float32),
            "f1aDR": f1aDR.astype(NPF8),
            "f1a2": f1a2.astype(NPF8),
            "f1bDR": f1bDR.astype(NPF8),
            "f1b2": f1b2.astype(NPF8),
            "f2full": f2full,
            "w3full": w3full,
            "afull": afull,
        })
    return in_maps, (f2_8, f2a, b2s, c0)


def kernel(local_inputs, labels, extras, DisM, AngleM,
           Wih, b_ih, b_hh, Wt, bt_ih, bt_hh,
           wp, bp, F1, b1, F2, b2, ff, bff,
           fuse1, biasf, Wout, biasout, a, _trace=False, _tmpdir=None):
    in_maps, (f2_8, f2a, b2s, c0) = prepare_inputs(
        local_inputs, labels, extras, DisM, AngleM,
        Wih, b_ih, b_hh, Wt, bt_ih, bt_hh, wp, bp, F1, b1, F2, b2,
        ff, bff, fuse1, biasf, Wout, biasout, a)
    nc = _get_nc(f2_8, f2a, b2s, c0)
    res = bass_utils.run_bass_kernel_spmd(
        nc, in_maps, core_ids=list(range(NCORES)), trace=_trace, tmpdir=_tmpdir)

    preds = np.empty((T, B, 1), np.float32)
    for cix in range(NCORES):
        out = res.results[cix]["preds"]  # [128, TLOC*4]: [p, tau*4+i4]
        o = out.reshape(128, TLOC, 4).transpose(1, 2, 0).reshape(TLOC, B)
        preds[cix * TLOC:(cix + 1) * TLOC, :, 0] = o

    labels_r = np.ascontiguousarray(
        np.transpose(np.asarray(labels), (1, 0, 2, 3)).reshape(T, B, 1))
    kernel._last_result = res
    return preds, labels_r

